# revision 1
# baseline (speedup 1.0000x reference)
"""Trainium2 Bass kernel for nn_Attention_80384607912675.

Multi-head attention (B=2, S=2048, D=1024, H=16, HD=64), fp32 reference.

Sharding (8 cores): data-parallel over batch (2) x tensor-parallel over heads
(4 head groups of 4 heads).  Core c handles batch c//4, heads [4*(c%4), 4*(c%4)+4).
wq/wk/wv split column-wise, wo split row-wise; the wo partial sums (and the
bias bo) are reduced on the host.

Per-core kernel (all matmuls bf16 with fp32 PSUM accumulation):
  QT/KT = (x @ wq/k + b)^T  stored head-major [256, 2048]
  V_aug = [x @ wv + bv | 1] stored natural    [2048, 4*(64+1)]  (ones column
                            per head folds the softmax row-sum into PV)
  per head pair hp (A/B), q-window qw (512 wide), kp-chunk c:
    S^T[kp, q]   = K_h^T (x) Q_h   (K=64; A,B packed side-by-side in one
                                    PSUM tile, row-tiled in the PE)
    P^T          = exp(S^T / 8)    (one ScalarE instr per A|B pair, ->bf16)
    [O^T; rowsum] += V_aug^T (x) P^T   (M=65, accumulated over c)
  O^T normalized by broadcast(1/rowsum) (PE K=1 broadcast + DVE multiply)
  out_partial = O_norm^T.T @ wo_c -> [2048, 1024] bf16 (heads 0-1 staged
  under the second attention pair, heads 2-3 added in the tail)

The exp (ScalarE) stream is the bottleneck (~134us busy); all PE-side work
(projections, normalization, output projection) is interleaved into its slack
via a static filler schedule, and the lead-in QK projections pipeline against
the streamed xT DMA chunks.
"""

import numpy as np

B, S, D, H = 2, 2048, 1024, 16
HD = D // H          # 64
HPC = 4              # heads per core
DHC = HPC * HD       # 256 head dims per core
KC = D // 128        # 8 contraction chunks
SB = S // 128        # 16 s blocks / kp chunks
VP = HPC * (HD + 1)  # 260: V storage pitch per s-chunk (ones col per head)
NC = 8               # cores
NQW = 4              # 512-wide q windows per head pair

_nc_cache = {}


def _build_bass(debug=False, with_bias=False):
    import concourse.mybir as mybir
    import concourse.tile as tile
    from concourse import bacc

    BF = mybir.dt.bfloat16
    F32 = mybir.dt.float32
    EXP = mybir.ActivationFunctionType.Exp

    nc = bacc.Bacc("TRN2")

    xT_d = nc.dram_tensor("xT", [D, S], BF, kind="ExternalInput")
    wq_d = nc.dram_tensor("wq_c", [D, DHC], BF, kind="ExternalInput")
    wk_d = nc.dram_tensor("wk_c", [D, DHC], BF, kind="ExternalInput")
    wv_d = nc.dram_tensor("wv_c", [D, DHC], BF, kind="ExternalInput")
    wo_d = nc.dram_tensor("wo_c", [DHC, D], BF, kind="ExternalInput")
    bias_d = nc.dram_tensor("bias3", [1, 3 * DHC], BF, kind="ExternalInput")
    out1_d = nc.dram_tensor("out1", [S, D], BF, kind="ExternalOutput")
    out2_d = nc.dram_tensor("out2", [S, D], BF, kind="ExternalOutput")
    if debug:
        dbg = {
            "qt": nc.dram_tensor("dbg_qt", [128, 2 * S], BF, kind="ExternalOutput"),
            "kt": nc.dram_tensor("dbg_kt", [128, 2 * S], BF, kind="ExternalOutput"),
            "v": nc.dram_tensor("dbg_v", [128, SB * VP], BF, kind="ExternalOutput"),
            "oun": nc.dram_tensor(
                "dbg_oun", [HD + 1, HPC * S], F32, kind="ExternalOutput"
            ),
            "onm": nc.dram_tensor("dbg_onm", [HD, HPC * S], BF, kind="ExternalOutput"),
            "onm2": nc.dram_tensor("dbg_onm2", [128, 2 * S], BF, kind="ExternalOutput"),
            "wo2": nc.dram_tensor("dbg_wo2", [128, 2 * D], BF, kind="ExternalOutput"),
            "ost": nc.dram_tensor("dbg_ost", [128, SB * D], BF, kind="ExternalOutput"),
        }

    with tile.TileContext(nc) as tc:
        with (
            tc.tile_pool(name="persist", bufs=1) as pp,
            tc.tile_pool(name="sc", bufs=2, space="PSUM") as scp,
            tc.tile_pool(name="oacc", bufs=1, space="PSUM") as opp,
            tc.tile_pool(name="pj", bufs=2, space="PSUM") as pjp,
            tc.tile_pool(name="pt", bufs=6) as ptp,
            tc.tile_pool(name="rc", bufs=2) as rcp,
            tc.tile_pool(name="bb", bufs=4) as bbp,
            tc.tile_pool(name="osb", bufs=6) as oup,
        ):
            xT_sb = pp.tile([128, KC * S], BF, tag="xT", name="xT_sb")
            wq_sb = pp.tile([128, KC * DHC], BF, tag="wq", name="wq_sb")
            wk_sb = pp.tile([128, KC * DHC], BF, tag="wk", name="wk_sb")
            wv_sb = pp.tile([128, KC * DHC], BF, tag="wv", name="wv_sb")
            wo_sb = pp.tile([128, 2 * D], BF, tag="wo", name="wo_sb")
            qt_sb = pp.tile([128, 2 * S], BF, tag="qt", name="qt_sb")
            kt_sb = pp.tile([128, 2 * S], BF, tag="kt", name="kt_sb")
            v_sb = pp.tile([128, SB * VP], BF, tag="v", name="v_sb")
            vt_sb = pp.tile([128, 2 * S], BF, tag="vt", name="vt_sb")
            ident = pp.tile([128, 128], BF, tag="ident", name="ident")
            oun_sb = pp.tile([HD + 1, HPC * S], F32, tag="oun", name="oun_sb")
            onm_sb = pp.tile([HD, HPC * S], BF, tag="onm", name="onm_sb")
            onm2_sb = pp.tile([128, 2 * S], BF, tag="onm2", name="onm2_sb")
            bias_sb = pp.tile([1, 3 * DHC], BF, tag="bias", name="bias_sb")
            ones16 = pp.tile([1, 512], BF, tag="ones16", name="ones16")

            # input DMAs: small weights first; xT streamed in 8 chunks that the
            # lead-in projections consume as they land; wo last (needed late)
            def load_w(w_sb, w_d):
                nc.sync.dma_start(
                    w_sb[:, :].rearrange("p (k d) -> p k d", d=DHC),
                    w_d[:, :].rearrange("(k p) d -> p k d", p=128),
                )

            def load_xt(k):
                nc.sync.dma_start(
                    xT_sb[:, k * S:(k + 1) * S], xT_d[k * 128:(k + 1) * 128, :]
                )

            load_w(wq_sb, wq_d)
            for k in range(4):
                load_xt(k)
            load_w(wk_sb, wk_d)
            for k in range(4, KC):
                load_xt(k)
            load_w(wv_sb, wv_d)
            nc.sync.dma_start(bias_sb[:, :], bias_d[:, :])
            nc.sync.dma_start(
                wo_sb[:, :].rearrange("r (p d) -> r p d", d=D),
                wo_d[:, :].rearrange("(p r) d -> r p d", r=128),
            )
            nc.vector.memset(ones16[:, :], 1.0)
            # ones columns of V_aug: preset everything to 1, V overwrites below
            nc.gpsimd.memset(v_sb[:, :], 1.0)
            from concourse.masks import make_identity
            make_identity(nc, ident[:, :])

            bq = bias_sb[0:1, 0:DHC]
            bk = bias_sb[0:1, DHC:2 * DHC]
            bv = bias_sb[0:1, 2 * DHC:3 * DHC]

            def qk_mm(ps, w_sb, p, nt, k):
                nc.tensor.matmul(
                    ps[:, :],
                    lhsT=w_sb[:, k * DHC + p * 128: k * DHC + (p + 1) * 128],
                    rhs=xT_sb[:, k * S + nt * 512: k * S + (nt + 1) * 512],
                    start=(k == 0),
                    stop=(k == KC - 1 and not with_bias),
                )

            def qk_fin(ps, dst, bias, p, nt, on_act=False):
                if with_bias:
                    nc.tensor.matmul(
                        ps[:, :],
                        lhsT=bias[:, p * 128:(p + 1) * 128],
                        rhs=ones16[0:1, :],
                        start=False,
                        stop=True,
                    )
                dslice = dst[:, p * S + nt * 512: p * S + (nt + 1) * 512]
                if on_act:
                    nc.scalar.copy(dslice, ps[:, :])
                else:
                    nc.vector.tensor_copy(dslice, ps[:, :])

            _qk_pending = {}

            def proj_qk_a(dst, w_sb, bias, p, nt):
                ps = pjp.tile([128, 512], F32, tag="pj", name=f"qk_{p}_{nt}")
                for k in range(KC // 2):
                    qk_mm(ps, w_sb, p, nt, k)
                _qk_pending[(p, nt, dst.tensor.name)] = ps

            def proj_qk_b(dst, w_sb, bias, p, nt):
                ps = _qk_pending.pop((p, nt, dst.tensor.name))
                for k in range(KC // 2, KC):
                    qk_mm(ps, w_sb, p, nt, k)
                qk_fin(ps, dst, bias, p, nt)

            def proj_vt(db, nt, pool=None, tag="pj"):
                """V^T[d-block db, s-window nt]: (wv^T x^T + bv) -> vt_sb bf16."""
                ps = (pool or pjp).tile([128, 512], F32, tag=tag, name=f"vt_{db}_{nt}")
                for k in range(KC):
                    nc.tensor.matmul(
                        ps[:, :],
                        lhsT=wv_sb[:, k * DHC + db * 128: k * DHC + (db + 1) * 128],
                        rhs=xT_sb[:, k * S + nt * 512: k * S + (nt + 1) * 512],
                        start=(k == 0),
                        stop=(k == KC - 1 and not with_bias),
                    )
                if with_bias:
                    nc.tensor.matmul(
                        ps[:, :],
                        lhsT=bv[:, db * 128:(db + 1) * 128],
                        rhs=ones16[0:1, :],
                        start=False,
                        stop=True,
                    )
                nc.vector.tensor_copy(
                    vt_sb[:, db * S + nt * 512: db * S + (nt + 1) * 512], ps[:, :]
                )

            def v_tp(sb, db):
                """Transpose V^T block (d-block db, s-chunk sb) into v_sb."""
                tp = pjp.tile([128, 128], BF, tag="pj", name=f"tp_{sb}_{db}")
                nc.tensor.transpose(
                    tp[:, :], vt_sb[:, db * S + sb * 128: db * S + (sb + 1) * 128],
                    ident[:, :],
                )
                dst3 = v_sb[
                    :, sb * VP + 2 * db * (HD + 1): sb * VP + (2 * db + 2) * (HD + 1)
                ].rearrange("p (h e) -> p h e", e=HD + 1)[:, :, 0:HD]
                nc.vector.tensor_copy(dst3, tp[:, :])

            def outproj_piece(sb, n, pair, out_dram, on_act=False):
                """Half s-block head-pair partial -> bf16 -> DMA."""
                ot = oup.tile([128, 512], BF, tag="osb", name=f"ot{pair}_{sb}_{n}")
                po = pjp.tile([128, 512], F32, tag="pj", name=f"po{pair}_{sb}_{n}")
                nc.tensor.matmul(
                    po[:, :],
                    lhsT=onm2_sb[:, pair * S + sb * 128: pair * S + (sb + 1) * 128],
                    rhs=wo_sb[:, pair * D + n * 512: pair * D + (n + 1) * 512],
                    start=True,
                    stop=True,
                )
                if on_act:
                    nc.scalar.copy(ot[:, :], po[:, :])
                else:
                    nc.vector.tensor_copy(ot[:, :], po[:, :])
                nc.sync.dma_start(
                    out_dram[sb * 128:(sb + 1) * 128, n * 512:(n + 1) * 512], ot[:, :]
                )

            def outproj1(sb, n):
                outproj_piece(sb, n, 0, out1_d)

            def outproj2(sb, on_act=False):
                for n in range(2):
                    outproj_piece(sb, n, 1, out2_d, on_act=on_act)

            # ---- lead-in: QT/KT p0 all nt, k-major across 4 psum slots so the
            # matmuls pipeline against the arriving xT chunks
            lead = [
                (qt_sb, wq_sb, bq, 0, 0, pjp, "pj"),
                (kt_sb, wk_sb, bk, 0, 0, pjp, "pj"),
                (kt_sb, wk_sb, bk, 0, 1, scp, "sc"),
                (qt_sb, wq_sb, bq, 0, 1, scp, "sc"),
                (None, wv_sb, bv, 0, 0, opp, "oacc"),  # V^T(0,0)
            ]
            lead_ps = [
                pool.tile([128, 512], F32, tag=tag, name=f"lead_{nt}_{tag}")
                for dst, w_sb, bias, p, nt, pool, tag in lead
            ]
            for k in range(KC):
                for (dst, w_sb, bias, p, nt, pool, tag), ps in zip(lead, lead_ps):
                    if dst is None:
                        nc.tensor.matmul(
                            ps[:, :],
                            lhsT=wv_sb[:, k * DHC + p * 128: k * DHC + (p + 1) * 128],
                            rhs=xT_sb[:, k * S + nt * 512: k * S + (nt + 1) * 512],
                            start=(k == 0),
                            stop=(k == KC - 1 and not with_bias),
                        )
                    else:
                        qk_mm(ps, w_sb, p, nt, k)
            for (dst, w_sb, bias, p, nt, pool, tag), ps in zip(lead, lead_ps):
                if dst is None:
                    if with_bias:
                        nc.tensor.matmul(
                            ps[:, :],
                            lhsT=bv[:, p * 128:(p + 1) * 128],
                            rhs=ones16[0:1, :],
                            start=False,
                            stop=True,
                        )
                    nc.scalar.copy(
                        vt_sb[:, p * S + nt * 512: p * S + (nt + 1) * 512], ps[:, :]
                    )
                else:
                    qk_fin(ps, dst, bias, p, nt, on_act=True)

            # ---- filler schedule: (hp, qw, c) -> deferred work emitted inside
            # the ACT-bound attention loop
            fillers = {}

            def add(hp, qw, c, fn):
                fillers.setdefault((hp, qw, c), []).append(fn)

            def add_qk(hp, qw, c, dst, w_sb, bias, p, nt):
                add(hp, qw, c, lambda: proj_qk_a(dst, w_sb, bias, p, nt))
                add(hp, qw, c + 1, lambda: proj_qk_b(dst, w_sb, bias, p, nt))

            for c in range(SB):  # heads 0,1 transposes JIT (vt block c//4 ready)
                add(0, 0, c, lambda c=c: v_tp(c, 0))
            add(0, 0, 1, lambda: proj_vt(0, 1))
            add(0, 0, 5, lambda: proj_vt(0, 2))
            add(0, 0, 9, lambda: proj_vt(0, 3))
            add_qk(0, 0, 2, kt_sb, wk_sb, bk, 0, 2)
            add_qk(0, 0, 6, kt_sb, wk_sb, bk, 0, 3)
            for nt in range(4):  # V^T heads 2,3 (needed from hp1)
                add(0, 1, 4 * nt, lambda nt=nt: proj_vt(1, nt))
            for i in range(8):
                add(0, 1, 2 * i + 1, lambda sb=i: v_tp(sb, 1))
                add(0, 2, 2 * i, lambda sb=i + 8: v_tp(sb, 1))
            add_qk(0, 1, 1, qt_sb, wq_sb, bq, 0, 2)
            add_qk(0, 1, 9, qt_sb, wq_sb, bq, 0, 3)
            add_qk(0, 2, 1, kt_sb, wk_sb, bk, 1, 0)
            add_qk(0, 2, 9, kt_sb, wk_sb, bk, 1, 1)
            add_qk(0, 3, 0, kt_sb, wk_sb, bk, 1, 2)
            add_qk(0, 3, 4, kt_sb, wk_sb, bk, 1, 3)
            add_qk(0, 3, 8, qt_sb, wq_sb, bq, 1, 0)
            add_qk(0, 3, 12, qt_sb, wq_sb, bq, 1, 1)
            add_qk(1, 0, 2, qt_sb, wq_sb, bq, 1, 2)
            add_qk(1, 0, 6, qt_sb, wq_sb, bq, 1, 3)
            for i in range(32):  # outproj stage 1 spread over hp1 qw0/qw1
                sb, n = divmod(i, 2)
                add(1, i // 16, i % 16, lambda sb=sb, n=n: outproj1(sb, n))
            for i in range(8):  # outproj stage 2 for sb 0..7 under hp1 qw2
                add(1, 2, 1 + 2 * (i % 8), lambda sb=i: outproj2(sb))
            for i in range(8, 12):  # sb 8..11 under hp1 qw3 (after its drains)
                add(1, 3, 4 + 2 * (i - 8), lambda sb=i: outproj2(sb))

            def drain_window(hp, qw, oacc, part):
                """Deferred per-window drain: park+recip / bcast+norm+relocate."""
                hA = 2 * hp
                oun4 = oun_sb[:, :].rearrange("p (h s) -> p h s", h=HPC)
                if part == 0:
                    nc.vector.tensor_copy(
                        oun4[0:HD + 1, hA:hA + 2, qw * 512:(qw + 1) * 512],
                        oacc[:, :],
                    )
                    return
                rs0 = rcp.tile([1, 1024], F32, tag="rs0", name=f"rs0_{hp}{qw}")
                nc.sync.dma_start(
                    rs0[0:1, :],
                    oun4[HD:HD + 1, hA:hA + 2, qw * 512:(qw + 1) * 512],
                )
                rc = rcp.tile([1, 1024], F32, tag="rc", name=f"rc_{hp}{qw}")
                nc.vector.reciprocal_approx_fast(out=rc[0:1, :], in_=rs0[0:1, :])
                for i in range(2):
                    h = 2 * hp + i
                    pb = bbp.tile([HD, 512], F32, tag="bb", name=f"bb_{hp}{qw}{i}")
                    nc.gpsimd.partition_broadcast(
                        pb[:, :], rc[0:1, i * 512:(i + 1) * 512]
                    )
                    qcol = h * S + qw * 512
                    nc.vector.tensor_mul(
                        onm_sb[0:HD, qcol:qcol + 512],
                        oun_sb[0:HD, qcol:qcol + 512],
                        pb[:, :],
                    )
                    nc.sync.dma_start(
                        onm2_sb[64 * i:64 * (i + 1), hp * S + qw * 512:
                                hp * S + (qw + 1) * 512],
                        onm_sb[0:HD, qcol:qcol + 512],
                    )

            # ---- attention
            pending_drain = []
            for hp in range(2):
                for qw in range(NQW):
                    oacc = opp.tile(
                        [HD + 1, 1024], F32, tag="oacc", name=f"o_{hp}_{qw}"
                    )
                    prev = None

                    def emit_pv(pt_t, c, oacc=oacc, hp=hp):
                        for i in range(2):
                            nc.tensor.matmul(
                                oacc[:, i * 512:(i + 1) * 512],
                                lhsT=v_sb[
                                    :, c * VP + (HD + 1) * (2 * hp + i):
                                    c * VP + (HD + 1) * (2 * hp + i + 1)
                                ],
                                rhs=pt_t[:, 512 * i:512 * (i + 1)],
                                start=(c == 0),
                                stop=(c == SB - 1),
                            )

                    for c in range(SB):
                        sc = scp.tile(
                            [128, 1024], F32, tag="sc", name=f"sc_{hp}{qw}{c}"
                        )
                        for i in range(2):  # head A | head B packed
                            nc.tensor.matmul(
                                sc[:, 512 * i:512 * (i + 1)],
                                lhsT=kt_sb[
                                    64 * i:64 * (i + 1),
                                    hp * S + c * 128: hp * S + (c + 1) * 128,
                                ],
                                rhs=qt_sb[
                                    64 * i:64 * (i + 1),
                                    hp * S + qw * 512: hp * S + (qw + 1) * 512,
                                ],
                                start=True,
                                stop=True,
                            )
                        pt_t = ptp.tile(
                            [128, 1024], BF, tag="pt", name=f"pt_{hp}{qw}{c}"
                        )
                        nc.scalar.activation(pt_t[:, :], sc[:, :], EXP, scale=0.125)
                        if pending_drain and c == 0:
                            drain_window(*pending_drain[0], 0)
                        elif pending_drain and c == 2:
                            drain_window(*pending_drain.pop(0), 1)
                        for fn in fillers.get((hp, qw, c), ()):
                            fn()
                        if prev is not None:
                            emit_pv(prev, c - 1)
                        prev = pt_t
                    emit_pv(prev, SB - 1)
                    pending_drain.append((hp, qw, oacc))

            while pending_drain:
                hp, qw, oacc = pending_drain.pop(0)
                drain_window(hp, qw, oacc, 0)
                drain_window(hp, qw, oacc, 1)
            for sb in range(12, SB):
                outproj2(sb, on_act=True)

            if debug:
                nc.sync.dma_start(dbg["qt"][:, :], qt_sb[:, :])
                nc.sync.dma_start(dbg["kt"][:, :], kt_sb[:, :])
                nc.sync.dma_start(dbg["v"][:, :], v_sb[:, :])
                nc.sync.dma_start(dbg["oun"][:, :], oun_sb[:, :])
                nc.sync.dma_start(dbg["onm"][:, :], onm_sb[:, :])
                nc.sync.dma_start(dbg["onm2"][:, :], onm2_sb[:, :])
                nc.sync.dma_start(dbg["wo2"][:, :], wo_sb[:, :])
                nc.sync.dma_start(dbg["ost"][:, :], ost_sb[:, :])

    nc.compile()
    return nc


def _get_nc(with_bias=False):
    if with_bias not in _nc_cache:
        _nc_cache[with_bias] = _build_bass(with_bias=with_bias)
    return _nc_cache[with_bias]


def _prepare_in_maps(x, wq, bq, wk, bk, wv, bv, wo):
    import ml_dtypes

    bf16 = ml_dtypes.bfloat16
    x = np.asarray(x, np.float32)
    wq, bq = np.asarray(wq, np.float32), np.asarray(bq, np.float32)
    wk, bk = np.asarray(wk, np.float32), np.asarray(bk, np.float32)
    wv, bv = np.asarray(wv, np.float32), np.asarray(bv, np.float32)
    wo = np.asarray(wo, np.float32)

    xT = [np.ascontiguousarray(x[b].T).astype(bf16) for b in range(B)]
    in_maps = []
    for c in range(NC):
        b, j = divmod(c, HPC)
        cs = slice(DHC * j, DHC * (j + 1))
        bias3 = np.concatenate([bq[cs], bk[cs], bv[cs]]).reshape(1, 3 * DHC).astype(bf16)
        in_maps.append(
            {
                "xT": xT[b],
                "wq_c": np.ascontiguousarray(wq[:, cs]).astype(bf16),
                "wk_c": np.ascontiguousarray(wk[:, cs]).astype(bf16),
                "wv_c": np.ascontiguousarray(wv[:, cs]).astype(bf16),
                "wo_c": np.ascontiguousarray(wo[cs, :]).astype(bf16),
                "bias3": np.ascontiguousarray(bias3),
            }
        )
    return in_maps


def _gather(parts, bo):
    bo = np.asarray(bo, np.float32)
    out = np.empty((B, S, D), np.float32)
    for b in range(B):
        acc = np.asarray(parts[HPC * b], np.float32)
        for j in range(1, HPC):
            acc = acc + np.asarray(parts[HPC * b + j], np.float32)
        out[b] = acc + bo
    return out


def kernel(x, wq, bq, wk, bk, wv, bv, wo, bo):
    from concourse import bass_utils

    in_maps = _prepare_in_maps(x, wq, bq, wk, bk, wv, bv, wo)
    with_bias = bool(
        np.any(np.asarray(bq)) or np.any(np.asarray(bk)) or np.any(np.asarray(bv))
    )
    res = bass_utils.run_bass_kernel_spmd(
        nc=_get_nc(with_bias), in_maps=in_maps, core_ids=list(range(NC))
    )
    parts = [
        np.asarray(r["out1"], np.float32) + np.asarray(r["out2"], np.float32)
        for r in res.results
    ]
    return _gather(parts, bo)



# revision 2
# speedup vs baseline: 1.1777x; 1.1777x over previous
"""Trainium2 Bass kernel for nn_Attention_80384607912675.

Multi-head attention (B=2, S=2048, D=1024, H=16, HD=64), fp32 reference.

Sharding (8 cores): data-parallel over batch (2) x tensor-parallel over heads
(4 head groups of 4 heads).  Core c handles batch c//4, heads [4*(c%4), 4*(c%4)+4).
wq/wk/wv split column-wise, wo split row-wise; the wo partial sums (and the
bias bo) are reduced on the host in fp32.

Per-core kernel (all matmuls bf16, fp32 PSUM accumulation):
  QT/KT = (x @ wq/k + b)^T   head-major [128 (2 heads x 64), 2048] per pair
  V     = x @ wv + bv        natural    [2048, 256] (xT as lhsT -> no transpose)
  per window w = (head pair hp, 512-wide q window qw), kp-chunk c:
    S^T[kp, (h, q)] = K_h^T (x) Q_h      packed [128, 1024] PSUM (A|B)
    P^T             = exp(S^T / 8)       one ACT instr -> bf16 SBUF
    O[q, (qc,h,hd)] += P^T(x)V chunks    [128 q, 64] tiles, full-partition PE
    rs[q, (qc,h)]   += P^T(x)1           rowsums, 1-wide matmuls
  drain: O+rs PSUM -> SBUF copy (frees the single O bank fast), reciprocal,
  per-partition normalize mul (DVE), PE transpose to O^T, out = O^T.T @ wo_c
  -> bf16 [2048, 1024] partial, DMA per [128, 512] tile.

The PE (tensor) engine is the bottleneck (~140us of matmul rows); the exp
stream on ACT (~133us) runs just under it.  All projection/PV/outproj work is
statically interleaved into the exp-paced window loop; PV runs lagged a few
chunks behind exp so the in-order PE stream never waits on V or the O-bank
drain.  xT streams in s-halves so the lead-in projections start ~5us in.
"""

import numpy as np

B, S, D, H = 2, 2048, 1024, 16
HD = D // H          # 64
HPC = 4              # heads per core
DHC = HPC * HD       # 256 head dims per core
KC = D // 128        # 8 contraction chunks
SB = S // 128        # 16 s blocks / kp chunks
NC = 8               # cores
NW = 8               # windows (2 head pairs x 4 q windows)

_nc_cache = {}


def _build_bass(with_bias=False):
    import concourse.mybir as mybir
    import concourse.tile as tile
    from concourse import bacc
    from concourse.masks import make_identity

    BF = mybir.dt.bfloat16
    F32 = mybir.dt.float32
    EXP = mybir.ActivationFunctionType.Exp

    nc = bacc.Bacc("TRN2")

    xT_d = nc.dram_tensor("xT", [D, S], BF, kind="ExternalInput")
    wq_d = nc.dram_tensor("wq_c", [D, DHC], BF, kind="ExternalInput")
    wk_d = nc.dram_tensor("wk_c", [D, DHC], BF, kind="ExternalInput")
    wv_d = nc.dram_tensor("wv_c", [D, DHC], BF, kind="ExternalInput")
    wo_d = nc.dram_tensor("wo_c", [DHC, D], BF, kind="ExternalInput")
    bias_d = nc.dram_tensor("bias3", [1, 3 * DHC], BF, kind="ExternalInput")
    out_d = nc.dram_tensor("out", [S, D], BF, kind="ExternalOutput")

    with tile.TileContext(nc) as tc:
        with (
            tc.tile_pool(name="persist", bufs=1) as pp,
            tc.tile_pool(name="sc", bufs=2, space="PSUM") as scp,
            tc.tile_pool(name="oacc", bufs=1, space="PSUM") as opp,
            tc.tile_pool(name="pj", bufs=2, space="PSUM") as pjp,
            tc.tile_pool(name="pt", bufs=12) as ptp,
            tc.tile_pool(name="osb", bufs=2) as osbp,
            tc.tile_pool(name="nrm", bufs=2) as nrmp,
            tc.tile_pool(name="rc", bufs=2) as rcp,
            tc.tile_pool(name="ot", bufs=4) as otp,
        ):
            xT_sb = pp.tile([128, KC * S], BF, tag="xT", name="xT_sb")
            wq_sb = pp.tile([128, KC * DHC], BF, tag="wq", name="wq_sb")
            wk_sb = pp.tile([128, KC * DHC], BF, tag="wk", name="wk_sb")
            wv_sb = pp.tile([128, KC * DHC], BF, tag="wv", name="wv_sb")
            wo_sb = pp.tile([128, 2 * D], BF, tag="wo", name="wo_sb")
            qt_sb = pp.tile([128, 2 * S], BF, tag="qt", name="qt_sb")
            kt_sb = pp.tile([128, 2 * S], BF, tag="kt", name="kt_sb")
            v_sb = pp.tile([128, SB * DHC], BF, tag="v", name="v_sb")
            onmT_sb = pp.tile([128, 2 * S], BF, tag="onmT", name="onmT_sb")
            ident = pp.tile([128, 128], BF, tag="ident", name="ident")
            bias_sb = pp.tile([1, 3 * DHC], BF, tag="bias", name="bias_sb")
            ones16 = pp.tile([1, 512], BF, tag="ones16", name="ones16")
            ones_col = pp.tile([128, 1], BF, tag="ones_col", name="ones_col")

            # ---- input DMAs: small weights first, xT streamed in s-halves
            # (16 DMAs) so nt0/nt1-dependent projections start early; wo last.
            def load_w(w_sb, w_d):
                nc.sync.dma_start(
                    w_sb[:, :].rearrange("p (k d) -> p k d", d=DHC),
                    w_d[:, :].rearrange("(k p) d -> p k d", p=128),
                )

            load_w(wk_sb, wk_d)
            load_w(wq_sb, wq_d)
            load_w(wv_sb, wv_d)
            for h in range(2):
                for k in range(KC):
                    nc.sync.dma_start(
                        xT_sb[:, k * S + h * 1024: k * S + (h + 1) * 1024],
                        xT_d[k * 128:(k + 1) * 128, h * 1024:(h + 1) * 1024],
                    )
            nc.sync.dma_start(bias_sb[:, :], bias_d[:, :])
            nc.sync.dma_start(
                wo_sb[:, :].rearrange("r (p d) -> r p d", d=D),
                wo_d[:, :].rearrange("(p r) d -> r p d", r=128),
            )
            nc.vector.memset(ones16[:, :], 1.0)
            nc.vector.memset(ones_col[:, :], 1.0)
            make_identity(nc, ident[:, :])

            bq = bias_sb[0:1, 0:DHC]
            bk = bias_sb[0:1, DHC:2 * DHC]
            bv = bias_sb[0:1, 2 * DHC:3 * DHC]

            # ---- Q/K projection tiles (p: head-pair block, nt: 512 s cols),
            # emitted in two 4-k-chunk halves to fit filler slots.
            pend = {}

            def qk_half(dst_sb, w_sb, bias, p, nt, half, eng="dve"):
                key = (dst_sb.tensor.name, p, nt)
                if half == 0:
                    ps = pjp.tile([128, 512], F32, tag="pj",
                                  name=f"qk_{key[0]}_{p}_{nt}")
                    pend[key] = ps
                    ks = range(0, KC // 2)
                else:
                    ps = pend.pop(key)
                    ks = range(KC // 2, KC)
                for k in ks:
                    nc.tensor.matmul(
                        ps[:, :],
                        lhsT=w_sb[:, k * DHC + p * 128: k * DHC + (p + 1) * 128],
                        rhs=xT_sb[:, k * S + nt * 512: k * S + (nt + 1) * 512],
                        start=(k == 0),
                        stop=(k == KC - 1 and not with_bias),
                    )
                if half == 1:
                    if with_bias:
                        nc.tensor.matmul(
                            ps[:, :],
                            lhsT=bias[:, p * 128:(p + 1) * 128],
                            rhs=ones16[0:1, :],
                            start=False, stop=True,
                        )
                    dst = dst_sb[:, p * S + nt * 512: p * S + (nt + 1) * 512]
                    if eng == "act":
                        nc.scalar.copy(dst, ps[:, :])
                    else:
                        nc.vector.tensor_copy(dst, ps[:, :])

            # ---- V pair tiles: pair j covers s-chunks 2j, 2j+1 in natural
            # layout (lhsT = xT s-slice, rhs = wv chunk).
            vpend = {}

            def v_bias_mms(ps):
                for h2 in range(2):
                    nc.tensor.matmul(
                        ps[:, h2 * 256:(h2 + 1) * 256],
                        lhsT=ones16[0:1, 0:128], rhs=bv[:, :],
                        start=False, stop=(h2 == 1),
                    )

            def v_half(j, half):
                if half == 0:
                    ps = pjp.tile([128, 512], F32, tag="pj", name=f"v_{j}")
                    vpend[j] = ps
                else:
                    ps = vpend.pop(j)
                sc_ = 2 * j + half
                for k in range(KC):
                    nc.tensor.matmul(
                        ps[:, half * 256:(half + 1) * 256],
                        lhsT=xT_sb[:, k * S + sc_ * 128: k * S + (sc_ + 1) * 128],
                        rhs=wv_sb[:, k * DHC: k * DHC + DHC],
                        start=(k == 0 and half == 0),
                        stop=(k == KC - 1 and half == 1 and not with_bias),
                    )
                if half == 1:
                    if with_bias:
                        v_bias_mms(ps)
                    nc.vector.tensor_copy(
                        v_sb[:, 2 * j * DHC: (2 * j + 2) * DHC], ps[:, :])

            # ---- per-window attention pieces
            otiles = {}
            pts = {}

            def emit_qk_exp(w, c):
                hp, qw = w // 4, w % 4
                sc = scp.tile([128, 1024], F32, tag="sc", name=f"sc_{w}_{c}")
                for i in range(2):
                    nc.tensor.matmul(
                        sc[:, 512 * i:512 * (i + 1)],
                        lhsT=kt_sb[64 * i:64 * (i + 1),
                                   hp * S + c * 128: hp * S + (c + 1) * 128],
                        rhs=qt_sb[64 * i:64 * (i + 1),
                                  hp * S + qw * 512: hp * S + (qw + 1) * 512],
                        start=True, stop=True,
                    )
                pt = ptp.tile([128, 1024], BF, tag="pt", name=f"pt_{w}_{c}")
                nc.scalar.activation(pt[:, :], sc[:, :], EXP, scale=0.125)
                pts[(w, c)] = pt

            def pv(w, c):
                hp = w // 4
                if c == 0:
                    otiles[w] = opp.tile([128, 520], F32, tag="oacc",
                                         name=f"o_{w}")
                O = otiles[w]
                pt = pts.pop((w, c))
                for qc in range(4):
                    for h in range(2):
                        first = (c == 0 and qc == 0 and h == 0)
                        last = (c == SB - 1 and qc == 3 and h == 1)
                        lh = pt[:, h * 512 + qc * 128: h * 512 + (qc + 1) * 128]
                        nc.tensor.matmul(
                            O[:, qc * 128 + h * 64: qc * 128 + h * 64 + 64],
                            lhsT=lh,
                            rhs=v_sb[:, c * DHC + (2 * hp + h) * 64:
                                     c * DHC + (2 * hp + h) * 64 + 64],
                            start=first, stop=last,
                        )
                        nc.tensor.matmul(
                            O[:, 512 + qc * 2 + h: 513 + qc * 2 + h],
                            lhsT=lh, rhs=ones_col[:, 0:1],
                            start=first, stop=last,
                        )

            osbs = {}
            nrms = {}

            def drain_a(w):
                osb = osbp.tile([128, 520], F32, tag="osb", name=f"osb_{w}")
                nc.vector.tensor_copy(osb[:, :], otiles.pop(w)[:, :])
                osbs[w] = osb

            def drain_b(w):
                osb = osbs.pop(w)
                rc = rcp.tile([128, 8], F32, tag="rc", name=f"rc_{w}")
                nc.vector.reciprocal_approx_fast(
                    out=rc[:, :], in_=osb[:, 512:520])
                nrm = nrmp.tile([128, 512], BF, tag="nrm", name=f"nrm_{w}")
                for qc in range(4):
                    for h in range(2):
                        col = qc * 128 + h * 64
                        nc.vector.tensor_scalar_mul(
                            nrm[:, col:col + 64], osb[:, col:col + 64],
                            rc[:, qc * 2 + h: qc * 2 + h + 1])
                nrms[w] = nrm

            def drain_tp(w, qc):
                hp, qw = w // 4, w % 4
                nrm = nrms[w]
                tp = pjp.tile([128, 128], BF, tag="pj", name=f"tp_{w}_{qc}")
                nc.tensor.transpose(
                    tp[:, :], nrm[:, qc * 128:(qc + 1) * 128], ident[:, :])
                nc.vector.tensor_copy(
                    onmT_sb[:, hp * S + qw * 512 + qc * 128:
                            hp * S + qw * 512 + (qc + 1) * 128], tp[:, :])
                if qc == 3:
                    del nrms[w]

            def outproj(qw, qc, n, eng="dve"):
                t = qw * 4 + qc
                po = pjp.tile([128, 512], F32, tag="pj", name=f"po_{t}_{n}")
                for hp in range(2):
                    nc.tensor.matmul(
                        po[:, :],
                        lhsT=onmT_sb[:, hp * S + qw * 512 + qc * 128:
                                     hp * S + qw * 512 + (qc + 1) * 128],
                        rhs=wo_sb[:, hp * D + n * 512: hp * D + (n + 1) * 512],
                        start=(hp == 0), stop=(hp == 1),
                    )
                ot = otp.tile([128, 512], BF, tag="ot", name=f"ot_{t}_{n}")
                if eng == "act":
                    nc.scalar.copy(ot[:, :], po[:, :])
                else:
                    nc.vector.tensor_copy(ot[:, :], po[:, :])
                nc.sync.dma_start(
                    out_d[t * 128:(t + 1) * 128, n * 512:(n + 1) * 512],
                    ot[:, :])

            # ---- lead-in: kt/qt (p0, nt0) + V pairs 0,1 pipelined against
            # the arriving xT halves; kt/qt finish tile-major so window 0
            # starts as early as possible.
            lt = scp.tile([128, 1024], F32, tag="sc", name="lead_ktqt")
            lv = [opp.tile([128, 520], F32, tag="oacc", name="lead_v01"),
                  pjp.tile([128, 512], F32, tag="pj", name="lead_v23")]

            def lead_mm(k):
                for half, (w_sb,) in enumerate([(wk_sb,), (wq_sb,)]):
                    nc.tensor.matmul(
                        lt[:, half * 512:(half + 1) * 512],
                        lhsT=w_sb[:, k * DHC: k * DHC + 128],
                        rhs=xT_sb[:, k * S: k * S + 512],
                        start=(k == 0),
                        stop=(k == KC - 1 and not with_bias),
                    )

            def lead_vmm(k, pair):
                ps = lv[pair]
                for h2 in range(2):
                    sc_ = 2 * pair + h2
                    nc.tensor.matmul(
                        ps[:, h2 * 256:(h2 + 1) * 256],
                        lhsT=xT_sb[:, k * S + sc_ * 128: k * S + (sc_ + 1) * 128],
                        rhs=wv_sb[:, k * DHC: k * DHC + DHC],
                        start=(k == 0 and h2 == 0),
                        stop=(k == KC - 1 and h2 == 1 and not with_bias),
                    )

            for k in range(3):
                lead_mm(k)
                lead_vmm(k, 0)
                lead_vmm(k, 1)
            for k in range(3, KC):
                lead_mm(k)
            if with_bias:
                for half, bias in enumerate([bk, bq]):
                    nc.tensor.matmul(
                        lt[:, half * 512:(half + 1) * 512],
                        lhsT=bias[:, 0:128], rhs=ones16[0:1, :],
                        start=False, stop=True,
                    )
            nc.scalar.copy(kt_sb[:, 0:512], lt[:, 0:512])
            nc.scalar.copy(qt_sb[:, 0:512], lt[:, 512:1024])

            def lead_v_rest(pair):
                for k in range(3, KC):
                    lead_vmm(k, pair)
                if with_bias:
                    v_bias_mms(lv[pair])
                nc.vector.tensor_copy(
                    v_sb[:, 2 * pair * DHC: (2 * pair + 2) * DHC],
                    lv[pair][:, 0:512])

            # ---- static schedule: slot (w, c) -> deferred emissions
            sched = {}

            def at(w, s, fn):
                sched.setdefault((w, s), []).append(fn)

            def qk_sched(w, s, dst, wt, bias, p, nt):
                at(w, s, lambda: qk_half(dst, wt, bias, p, nt, 0))
                at(w, s + 1, lambda: qk_half(dst, wt, bias, p, nt, 1))

            def v_sched(w, s, j):
                at(w, s, lambda: v_half(j, 0))
                at(w, s + 1, lambda: v_half(j, 1))

            # W0 fillers: finish lead V, kt nt1-3 (due at c=4/8/12),
            # qt nt1 (due W1), V pairs 2-4.
            at(0, 0, lambda: lead_v_rest(0))
            at(0, 1, lambda: lead_v_rest(1))
            qk_sched(0, 2, kt_sb, wk_sb, bk, 0, 1)
            v_sched(0, 4, 2)
            qk_sched(0, 6, kt_sb, wk_sb, bk, 0, 2)
            v_sched(0, 8, 3)
            qk_sched(0, 10, kt_sb, wk_sb, bk, 0, 3)
            qk_sched(0, 12, qt_sb, wq_sb, bq, 0, 1)
            v_sched(0, 14, 4)
            # W1: V pairs 5-7, qt nt2
            v_sched(1, 0, 5)
            v_sched(1, 2, 6)
            v_sched(1, 4, 7)
            qk_sched(1, 6, qt_sb, wq_sb, bq, 0, 2)
            # W2: kt p1 nt0-1, qt nt3
            qk_sched(2, 0, kt_sb, wk_sb, bk, 1, 0)
            qk_sched(2, 2, qt_sb, wq_sb, bq, 0, 3)
            qk_sched(2, 4, kt_sb, wk_sb, bk, 1, 1)
            # W3: kt p1 nt2-3, qt p1 nt0
            qk_sched(3, 0, kt_sb, wk_sb, bk, 1, 2)
            qk_sched(3, 2, kt_sb, wk_sb, bk, 1, 3)
            qk_sched(3, 4, qt_sb, wq_sb, bq, 1, 0)
            # W4: qt p1 nt1-3
            qk_sched(4, 0, qt_sb, wq_sb, bq, 1, 1)
            qk_sched(4, 2, qt_sb, wq_sb, bq, 1, 2)
            qk_sched(4, 4, qt_sb, wq_sb, bq, 1, 3)

            # PV schedule.  W0: PV(0, 0..7) lagged inside W0; the rest of
            # window w's PV spills into window w+1 slots 0..7, then drain.
            at(0, 7, lambda: pv(0, 0))
            at(0, 7, lambda: pv(0, 1))
            for c in range(2, 8):
                at(0, 6 + c, lambda c=c: pv(0, c))

            for w in range(1, NW):
                for i in range(8):     # PV spill of previous window
                    at(w, max(i, 2) if w == 1 else i,
                       lambda w=w, i=i: pv(w - 1, 8 + i))
                at(w, 8, lambda w=w: drain_a(w - 1))
                at(w, 9, lambda w=w: drain_b(w - 1))
                at(w, 10, lambda w=w: pv(w, 0))
                at(w, 10, lambda w=w: pv(w, 1))
                for c in range(2, 8):
                    at(w, min(8 + c, 15), lambda w=w, c=c: pv(w, c))
                if w - 1 < 4:
                    for qc in range(4):
                        at(w, 11 + qc, lambda w=w, qc=qc: drain_tp(w - 1, qc))
                else:
                    # stagger transpose -> outproj of q-window (w-1)%4
                    qw = (w - 1) % 4
                    for qc in range(4):
                        at(w, 11 + qc, lambda w=w, qc=qc: drain_tp(w - 1, qc))
                        if qc > 0:
                            at(w, 11 + qc, lambda qw=qw, qc=qc: outproj(qw, qc - 1, 0))
                            at(w, 11 + qc, lambda qw=qw, qc=qc: outproj(qw, qc - 1, 1))
                    at(w, 15, lambda qw=qw: outproj(qw, 3, 0))
                    at(w, 15, lambda qw=qw: outproj(qw, 3, 1))

            # ---- main loop
            for w in range(NW):
                for c in range(SB):
                    emit_qk_exp(w, c)
                    for fn in sched.get((w, c), ()):
                        fn()

            # ---- tail: finish window 7
            for i in range(8):
                pv(7, 8 + i)
            drain_a(7)
            drain_b(7)
            for qc in range(4):
                drain_tp(7, qc)
                if qc > 0:
                    outproj(3, qc - 1, 0, eng="act")
                    outproj(3, qc - 1, 1, eng="dve")
            outproj(3, 3, 0, eng="act")
            outproj(3, 3, 1, eng="dve")

    nc.compile()
    return nc


def _get_nc(with_bias=False):
    if with_bias not in _nc_cache:
        _nc_cache[with_bias] = _build_bass(with_bias=with_bias)
    return _nc_cache[with_bias]


def _prepare_in_maps(x, wq, bq, wk, bk, wv, bv, wo):
    import ml_dtypes

    bf16 = ml_dtypes.bfloat16
    x = np.asarray(x, np.float32)
    wq, bq = np.asarray(wq, np.float32), np.asarray(bq, np.float32)
    wk, bk = np.asarray(wk, np.float32), np.asarray(bk, np.float32)
    wv, bv = np.asarray(wv, np.float32), np.asarray(bv, np.float32)
    wo = np.asarray(wo, np.float32)

    xT = [np.ascontiguousarray(x[b].T).astype(bf16) for b in range(B)]
    in_maps = []
    for c in range(NC):
        b, j = divmod(c, HPC)
        cs = slice(DHC * j, DHC * (j + 1))
        bias3 = np.concatenate([bq[cs], bk[cs], bv[cs]]).reshape(1, 3 * DHC).astype(bf16)
        in_maps.append(
            {
                "xT": xT[b],
                "wq_c": np.ascontiguousarray(wq[:, cs]).astype(bf16),
                "wk_c": np.ascontiguousarray(wk[:, cs]).astype(bf16),
                "wv_c": np.ascontiguousarray(wv[:, cs]).astype(bf16),
                "wo_c": np.ascontiguousarray(wo[cs, :]).astype(bf16),
                "bias3": np.ascontiguousarray(bias3),
            }
        )
    return in_maps


def _gather(parts, bo):
    bo = np.asarray(bo, np.float32)
    out = np.empty((B, S, D), np.float32)
    for b in range(B):
        acc = np.asarray(parts[HPC * b], np.float32)
        for j in range(1, HPC):
            acc = acc + np.asarray(parts[HPC * b + j], np.float32)
        out[b] = acc + bo
    return out


def kernel(x, wq, bq, wk, bk, wv, bv, wo, bo):
    from concourse import bass_utils

    in_maps = _prepare_in_maps(x, wq, bq, wk, bk, wv, bv, wo)
    with_bias = bool(
        np.any(np.asarray(bq)) or np.any(np.asarray(bk)) or np.any(np.asarray(bv))
    )
    res = bass_utils.run_bass_kernel_spmd(
        nc=_get_nc(with_bias), in_maps=in_maps, core_ids=list(range(NC))
    )
    parts = [np.asarray(r["out"], np.float32) for r in res.results]
    return _gather(parts, bo)


# revision 42
# speedup vs baseline: 1.2119x; 1.0290x over previous
"""Trainium2 Bass kernel for nn_Attention_80384607912675.

Multi-head attention (B=2, S=2048, D=1024, H=16, HD=64), fp32 reference.

Sharding (8 cores): data-parallel over batch (2) x tensor-parallel over heads
(4 head groups of 4 heads).  Core c handles batch c//4, heads [4*(c%4), 4*(c%4)+4).
wq/wk/wv split column-wise, wo split row-wise; the wo partial sums (and the
bias bo) are reduced on the host in fp32.

Per-core kernel (all matmuls bf16, fp32 PSUM accumulation):
  QT/KT = (x @ wq/k + b)^T   head-major [128 (2 heads x 64), 2048] per pair
  V     = x @ wv + bv        natural    [2048, 256] (xT as lhsT -> no transpose)
  per window w = (head pair hp, 512-wide q window qw), kp-chunk c:
    S^T[kp, (h, q)] = K_h^T (x) Q_h      packed [128, 1024] PSUM (A|B)
    P^T             = exp(S^T / 8)       one ACT instr -> bf16 SBUF
    O[q, (qc,h,hd)] += P^T(x)V chunks    [128 q, 64] tiles, full-partition PE
    rs[q, (qc,h)]   += P^T(x)1           rowsums, 1-wide matmuls
  drain: O+rs PSUM -> SBUF copy (frees the single O bank fast), reciprocal,
  per-partition normalize mul (DVE), PE transpose to O^T, out = O^T.T @ wo_c
  -> bf16 [2048, 1024] partial, DMA per [128, 512] tile.

The PE (tensor) engine is the bottleneck (~140us of matmul rows); the exp
stream on ACT (~133us) runs just under it.  All projection/PV/outproj work is
statically interleaved into the exp-paced window loop; PV runs lagged a few
chunks behind exp so the in-order PE stream never waits on V or the O-bank
drain.  xT streams in s-halves so the lead-in projections start ~5us in.
"""

import numpy as np

B, S, D, H = 2, 2048, 1024, 16
HD = D // H          # 64
HPC = 4              # heads per core
DHC = HPC * HD       # 256 head dims per core
KC = D // 128        # 8 contraction chunks
SB = S // 128        # 16 s blocks / kp chunks
NC = 8               # cores
NW = 8               # windows (2 head pairs x 4 q windows)

_nc_cache = {}


def _build_bass(with_bias=False):
    import concourse.mybir as mybir
    import concourse.tile as tile
    from concourse import bacc
    from concourse.masks import make_identity

    BF = mybir.dt.bfloat16
    F32 = mybir.dt.float32
    EXP = mybir.ActivationFunctionType.Exp

    nc = bacc.Bacc("TRN2")

    xT_d = nc.dram_tensor("xT", [D, S], BF, kind="ExternalInput")
    wq_d = nc.dram_tensor("wq_c", [D, DHC], BF, kind="ExternalInput")
    wk_d = nc.dram_tensor("wk_c", [D, DHC], BF, kind="ExternalInput")
    wv_d = nc.dram_tensor("wv_c", [D, DHC], BF, kind="ExternalInput")
    wo_d = nc.dram_tensor("wo_c", [DHC, D], BF, kind="ExternalInput")
    bias_d = nc.dram_tensor("bias3", [1, 3 * DHC], BF, kind="ExternalInput")
    out_d = nc.dram_tensor("out", [S, D], BF, kind="ExternalOutput")
    dbg_qt = nc.dram_tensor("dbg_qt", [128, 2 * S], BF, kind="ExternalOutput")
    dbg_kt = nc.dram_tensor("dbg_kt", [128, 2 * S], BF, kind="ExternalOutput")
    dbg_v = nc.dram_tensor("dbg_v", [128, SB * DHC], BF, kind="ExternalOutput")
    dbg_onmT = nc.dram_tensor("dbg_onmT", [128, 2 * S], BF,
                              kind="ExternalOutput")

    with tile.TileContext(nc) as tc:
        with (
            tc.tile_pool(name="persist", bufs=1) as pp,
            tc.tile_pool(name="sc", bufs=2, space="PSUM") as scp,
            tc.tile_pool(name="oacc", bufs=1, space="PSUM") as opp,
            tc.tile_pool(name="pj", bufs=2, space="PSUM") as pjp,
            tc.tile_pool(name="pt", bufs=12) as ptp,
            tc.tile_pool(name="osb", bufs=2) as osbp,
            tc.tile_pool(name="nrm", bufs=2) as nrmp,
            tc.tile_pool(name="rc", bufs=2) as rcp,
            tc.tile_pool(name="ot", bufs=4) as otp,
        ):
            xT_sb = pp.tile([128, KC * S], BF, tag="xT", name="xT_sb")
            wq_sb = pp.tile([128, KC * DHC], BF, tag="wq", name="wq_sb")
            wk_sb = pp.tile([128, KC * DHC], BF, tag="wk", name="wk_sb")
            wv_sb = pp.tile([128, KC * DHC], BF, tag="wv", name="wv_sb")
            wo_sb = pp.tile([128, 2 * D], BF, tag="wo", name="wo_sb")
            qt_sb = pp.tile([128, 2 * S], BF, tag="qt", name="qt_sb")
            kt_sb = pp.tile([128, 2 * S], BF, tag="kt", name="kt_sb")
            v_sb = pp.tile([128, SB * DHC], BF, tag="v", name="v_sb")
            onmT_sb = pp.tile([128, 2 * S], BF, tag="onmT", name="onmT_sb")
            ident = pp.tile([128, 128], BF, tag="ident", name="ident")
            bias_sb = pp.tile([1, 3 * DHC], BF, tag="bias", name="bias_sb")
            ones16 = pp.tile([1, 512], BF, tag="ones16", name="ones16")
            ones_col = pp.tile([128, 1], BF, tag="ones_col", name="ones_col")

            # ---- input DMAs: small weights first, xT streamed in s-halves
            # (16 DMAs) so nt0/nt1-dependent projections start early; wo last.
            def load_w(w_sb, w_d):
                nc.sync.dma_start(
                    w_sb[:, :].rearrange("p (k d) -> p k d", d=DHC),
                    w_d[:, :].rearrange("(k p) d -> p k d", p=128),
                )

            def load_xt(k, h):
                nc.sync.dma_start(
                    xT_sb[:, k * S + h * 1024: k * S + (h + 1) * 1024],
                    xT_d[k * 128:(k + 1) * 128, h * 1024:(h + 1) * 1024],
                )

            load_w(wk_sb, wk_d)
            load_xt(0, 0)
            load_xt(1, 0)
            load_w(wq_sb, wq_d)
            for k in range(2, KC):
                load_xt(k, 0)
            load_w(wv_sb, wv_d)
            for k in range(KC):
                load_xt(k, 1)
            nc.sync.dma_start(bias_sb[:, :], bias_d[:, :])
            nc.sync.dma_start(
                wo_sb[:, :].rearrange("r (p d) -> r p d", d=D),
                wo_d[:, :].rearrange("(p r) d -> r p d", r=128),
            )
            nc.vector.memset(ones16[:, :], 1.0)
            nc.vector.memset(ones_col[:, :], 1.0)
            make_identity(nc, ident[:, :])

            bq = bias_sb[0:1, 0:DHC]
            bk = bias_sb[0:1, DHC:2 * DHC]
            bv = bias_sb[0:1, 2 * DHC:3 * DHC]

            # ---- Q/K projection tiles (p: head-pair block, nt: 512 s cols),
            # emitted in four 2-k-chunk quarters so no single filler slot
            # exceeds the exp budget (locally PE-stalled exp slots are never
            # recovered).
            pend = {}

            def qk_quarter(dst_sb, w_sb, bias, p, nt, q, eng="dve"):
                key = (dst_sb.tensor.name, p, nt)
                if q == 0:
                    ps = pjp.tile([128, 512], F32, tag="pj",
                                  name=f"qk_{key[0]}_{p}_{nt}")
                    pend[key] = ps
                else:
                    ps = pend[key]
                for k in (2 * q, 2 * q + 1):
                    nc.tensor.matmul(
                        ps[:, :],
                        lhsT=w_sb[:, k * DHC + p * 128: k * DHC + (p + 1) * 128],
                        rhs=xT_sb[:, k * S + nt * 512: k * S + (nt + 1) * 512],
                        start=(k == 0),
                        stop=(k == KC - 1 and not with_bias),
                    )
                if q == 3:
                    del pend[key]
                    if with_bias:
                        nc.tensor.matmul(
                            ps[:, :],
                            lhsT=bias[:, p * 128:(p + 1) * 128],
                            rhs=ones16[0:1, :],
                            start=False, stop=True,
                        )
                    dst = dst_sb[:, p * S + nt * 512: p * S + (nt + 1) * 512]
                    if eng == "act":
                        nc.scalar.copy(dst, ps[:, :])
                    else:
                        nc.vector.tensor_copy(dst, ps[:, :])

            # ---- V pair tiles: pair j covers s-chunks 2j, 2j+1 in natural
            # layout (lhsT = xT s-slice, rhs = wv chunk), in 4-matmul
            # quarters.  Pairs 0/1 run in the lead psum slots (lv).
            vpend = {}

            def v_bias_mms(ps):
                for h2 in range(2):
                    nc.tensor.matmul(
                        ps[:, h2 * 256:(h2 + 1) * 256],
                        lhsT=ones16[0:1, 0:128], rhs=bv[:, :],
                        start=False, stop=(h2 == 1),
                    )

            def v_quarter(j, q):
                if j < 2:
                    ps = lv[j]
                elif q == 0:
                    ps = pjp.tile([128, 512], F32, tag="pj", name=f"v_{j}")
                    vpend[j] = ps
                else:
                    ps = vpend[j]
                half = q // 2
                sc_ = 2 * j + half
                for k in range(4 * (q % 2), 4 * (q % 2) + 4):
                    nc.tensor.matmul(
                        ps[:, half * 256:(half + 1) * 256],
                        lhsT=xT_sb[:, k * S + sc_ * 128: k * S + (sc_ + 1) * 128],
                        rhs=wv_sb[:, k * DHC: k * DHC + DHC],
                        start=(k == 0 and half == 0),
                        stop=(k == KC - 1 and half == 1 and not with_bias),
                    )
                if q == 3:
                    vpend.pop(j, None)
                    if with_bias:
                        v_bias_mms(ps)
                    nc.vector.tensor_copy(
                        v_sb[:, 2 * j * DHC: (2 * j + 2) * DHC],
                        ps[:, 0:512])

            # ---- per-window attention pieces
            otiles = {}
            pts = {}

            def emit_qk_exp(w, c):
                hp, qw = w // 4, w % 4
                sc = scp.tile([128, 1024], F32, tag="sc", name=f"sc_{w}_{c}")
                for i in range(2):
                    nc.tensor.matmul(
                        sc[:, 512 * i:512 * (i + 1)],
                        lhsT=kt_sb[64 * i:64 * (i + 1),
                                   hp * S + c * 128: hp * S + (c + 1) * 128],
                        rhs=qt_sb[64 * i:64 * (i + 1),
                                  hp * S + qw * 512: hp * S + (qw + 1) * 512],
                        start=True, stop=True,
                    )
                pt = ptp.tile([128, 1024], BF, tag="pt", name=f"pt_{w}_{c}")
                nc.scalar.activation(pt[:, :], sc[:, :], EXP, scale=0.125)
                pts[(w, c)] = pt

            def pv(w, c):
                hp = w // 4
                if c == 0:
                    otiles[w] = opp.tile([128, 520], F32, tag="oacc",
                                         name=f"o_{w}")
                O = otiles[w]
                pt = pts.pop((w, c))
                for qc in range(4):
                    for h in range(2):
                        first = (c == 0 and qc == 0 and h == 0)
                        last = (c == SB - 1 and qc == 3 and h == 1)
                        lh = pt[:, h * 512 + qc * 128: h * 512 + (qc + 1) * 128]
                        nc.tensor.matmul(
                            O[:, qc * 128 + h * 64: qc * 128 + h * 64 + 64],
                            lhsT=lh,
                            rhs=v_sb[:, c * DHC + (2 * hp + h) * 64:
                                     c * DHC + (2 * hp + h) * 64 + 64],
                            start=first, stop=last,
                        )
                        nc.tensor.matmul(
                            O[:, 512 + qc * 2 + h: 513 + qc * 2 + h],
                            lhsT=lh, rhs=ones_col[:, 0:1],
                            start=first, stop=last,
                        )

            osbs = {}
            nrms = {}

            def drain_a(w):
                osb = osbp.tile([128, 520], F32, tag="osb", name=f"osb_{w}")
                nc.vector.tensor_copy(osb[:, :], otiles.pop(w)[:, :])
                osbs[w] = osb

            def drain_b(w):
                osb = osbs.pop(w)
                rc = rcp.tile([128, 8], F32, tag="rc", name=f"rc_{w}")
                nc.vector.reciprocal_approx_fast(
                    out=rc[:, :], in_=osb[:, 512:520])
                nrm = nrmp.tile([128, 512], BF, tag="nrm", name=f"nrm_{w}")
                for qc in range(4):
                    for h in range(2):
                        col = qc * 128 + h * 64
                        nc.vector.tensor_scalar_mul(
                            nrm[:, col:col + 64], osb[:, col:col + 64],
                            rc[:, qc * 2 + h: qc * 2 + h + 1])
                nrms[w] = nrm

            def drain_tp(w, qc):
                hp, qw = w // 4, w % 4
                nrm = nrms[w]
                tp = pjp.tile([128, 128], BF, tag="pj", name=f"tp_{w}_{qc}")
                nc.tensor.transpose(
                    tp[:, :], nrm[:, qc * 128:(qc + 1) * 128], ident[:, :])
                nc.vector.tensor_copy(
                    onmT_sb[:, hp * S + qw * 512 + qc * 128:
                            hp * S + qw * 512 + (qc + 1) * 128], tp[:, :])
                if qc == 3:
                    del nrms[w]

            def outproj(qw, qc, n, eng="dve"):
                t = qw * 4 + qc
                po = pjp.tile([128, 512], F32, tag="pj", name=f"po_{t}_{n}")
                for hp in range(2):
                    nc.tensor.matmul(
                        po[:, :],
                        lhsT=onmT_sb[:, hp * S + qw * 512 + qc * 128:
                                     hp * S + qw * 512 + (qc + 1) * 128],
                        rhs=wo_sb[:, hp * D + n * 512: hp * D + (n + 1) * 512],
                        start=(hp == 0), stop=(hp == 1),
                    )
                ot = otp.tile([128, 512], BF, tag="ot", name=f"ot_{t}_{n}")
                if eng == "act":
                    nc.scalar.copy(ot[:, :], po[:, :])
                else:
                    nc.vector.tensor_copy(ot[:, :], po[:, :])
                nc.sync.dma_start(
                    out_d[t * 128:(t + 1) * 128, n * 512:(n + 1) * 512],
                    ot[:, :])

            # ---- lead-in: kt/qt (p0, nt0) + V pairs 0,1 pipelined against
            # the arriving xT halves; kt/qt finish first so window 0 starts
            # as early as possible.  Dummy identity transposes keep the PE
            # continuously busy from t~0 so the pstate ramp (full speed after
            # 3us of uninterrupted execution) is burned during the input DMA
            # instead of doubling every lead matmul.
            def ramp(n):
                for _ in range(n):
                    nc.tensor.transpose(
                        dummy_bf[:, :], ident[:, :], ident[:, :])

            dummy_bf = pjp.tile([128, 128], BF, tag="pj", name="dummy_bf")
            lt = scp.tile([128, 1024], F32, tag="sc", name="lead_ktqt")
            lv = [opp.tile([128, 520], F32, tag="oacc", name="lead_v01"),
                  pjp.tile([128, 512], F32, tag="pj", name="lead_v23")]

            def lead_mm(k):
                for half, (w_sb,) in enumerate([(wk_sb,), (wq_sb,)]):
                    nc.tensor.matmul(
                        lt[:, half * 512:(half + 1) * 512],
                        lhsT=w_sb[:, k * DHC: k * DHC + 128],
                        rhs=xT_sb[:, k * S: k * S + 512],
                        start=(k == 0),
                        stop=(k == KC - 1 and not with_bias),
                    )

            def lead_vmm(k, pair):
                ps = lv[pair]
                for h2 in range(2):
                    sc_ = 2 * pair + h2
                    nc.tensor.matmul(
                        ps[:, h2 * 256:(h2 + 1) * 256],
                        lhsT=xT_sb[:, k * S + sc_ * 128: k * S + (sc_ + 1) * 128],
                        rhs=wv_sb[:, k * DHC: k * DHC + DHC],
                        start=(k == 0 and h2 == 0),
                        stop=(k == KC - 1 and h2 == 1 and not with_bias),
                    )

            ramp(17)
            for k in range(KC):
                lead_mm(k)
                if k < KC - 1:
                    ramp(2 if k < 4 else 8)
            if with_bias:
                for half, bias in enumerate([bk, bq]):
                    nc.tensor.matmul(
                        lt[:, half * 512:(half + 1) * 512],
                        lhsT=bias[:, 0:128], rhs=ones16[0:1, :],
                        start=False, stop=True,
                    )
            nc.scalar.copy(kt_sb[:, 0:512], lt[:, 0:512])
            nc.vector.tensor_copy(qt_sb[:, 0:512], lt[:, 512:1024])

            # ---- schedule ----
            # Mandatory per-slot items (PV cadence, drains, transposes) are
            # placed at fixed global slots g = 16*w + c.  Everything else
            # (projection/V quarters, outproj pieces) lives in a strict-FIFO
            # unit queue packed greedily against each slot's leftover PE
            # budget, with earliest-slot gates and force-emit deadlines.
            sched = {}
            mcost = {}

            def at(g, fn, cost=0):
                sched.setdefault(g, []).append(fn)
                mcost[g] = mcost.get(g, 0) + cost

            # PV cadence: PV(w, c) lags exp by ~12 slots; the single O bank
            # serializes windows (drain_a frees it).  Last two windows
            # compress so the tail stays short.
            for w in range(NW):
                for c in range(SB):
                    if w < 6:
                        g = 16 * w + 12 + c
                    elif w == 6:
                        g = 108 + c if c < 8 else 112 + (c - 8) // 2
                    else:
                        g = (120 + c if c < 4 else
                             124 + (c - 4) // 2 if c < 12 else 200)
                    at(g, lambda w=w, c=c: pv(w, c), 240)
            for w in range(NW - 1):
                if w < 6:
                    ga, gtp = 16 * w + 27, 16 * w + 30
                else:
                    ga, gtp = 115, 118
                at(ga, lambda w=w: drain_a(w))
                at(ga + 1, lambda w=w: drain_b(w))
                for qc in range(4):
                    at(gtp + qc // 2, lambda w=w, qc=qc: drain_tp(w, qc), 60)

            # filler unit queue
            units = []

            def unit(e, dls, cost, mk):
                for q, d in enumerate(dls):
                    units.append((e[q] if isinstance(e, list) else e, d, cost,
                                  mk(q)))

            def mk_qk(dst, wt, bias, p, nt, eng="dve"):
                return lambda q: (lambda: qk_quarter(dst, wt, bias, p, nt, q,
                                                     eng))

            def mk_v(j):
                return lambda q: (lambda: v_quarter(j, q))

            def mk_po(qw, qc, n, eng="dve"):
                return lambda q: (lambda: outproj(qw, qc, n, eng))

            unit(0, [0, 1, 2, 3], 430, mk_qk(kt_sb, wk_sb, bk, 0, 1, "act"))
            unit(1, [3, 4, 5, 6], 430, mk_v(0))
            unit([1, 2, 3, 5], [4, 5, 6, 7], 430,
                 mk_qk(kt_sb, wk_sb, bk, 0, 2, "act"))
            unit(1, [5, 6, 7, 8], 430, mk_v(1))
            unit(0, [8, 9, 10, 11], 430, mk_v(2))
            unit([1, 2, 3, 5], [8, 9, 10, 11], 430,
                 mk_qk(kt_sb, wk_sb, bk, 0, 3, "act"))
            unit(0, [10, 11, 12, 13], 430, mk_v(3))
            unit(0, [12, 13, 14, 15], 430, mk_qk(qt_sb, wq_sb, bq, 0, 1, "act"))
            unit([3, 5, 5, 5], [14, 15, 16, 17], 430, mk_v(4))
            unit([4, 5, 5, 6], [18, 19, 20, 21], 430, mk_v(5))
            unit([4, 5, 6, 6], [20, 21, 22, 23], 430, mk_v(6))
            unit([5, 6, 6, 7], [22, 23, 24, 25], 430, mk_v(7))
            unit(0, [28, 29, 30, 31], 430, mk_qk(qt_sb, wq_sb, bq, 0, 2))
            unit(0, [44, 45, 46, 47], 430, mk_qk(qt_sb, wq_sb, bq, 0, 3))
            unit(0, [60, 61, 62, 63], 430, mk_qk(kt_sb, wk_sb, bk, 1, 0))
            unit(0, [64, 65, 66, 67], 430, mk_qk(kt_sb, wk_sb, bk, 1, 1))
            unit(0, [68, 69, 70, 71], 430, mk_qk(kt_sb, wk_sb, bk, 1, 2))
            unit(0, [72, 73, 74, 75], 430, mk_qk(kt_sb, wk_sb, bk, 1, 3))
            unit(0, [60, 61, 62, 63], 430, mk_qk(qt_sb, wq_sb, bq, 1, 0))
            unit(0, [76, 77, 78, 79], 430, mk_qk(qt_sb, wq_sb, bq, 1, 1))
            unit([88, 89, 90, 91], [92, 93, 94, 95], 430, mk_qk(qt_sb, wq_sb, bq, 1, 2))
            for i in range(8):
                units.append((96, 118 + i, 430, (lambda i=i: outproj(
                    0, i // 2, i % 2))))
            unit([100, 101, 102, 103], [108, 109, 110, 111], 430, mk_qk(qt_sb, wq_sb, bq, 1, 3))
            for i in range(8):
                units.append((112, 113 + i, 430, (lambda i=i: outproj(
                    1, i // 2, i % 2))))
            for i in range(8):
                units.append((120, 120 + i, 430, (lambda i=i: outproj(
                    2, i // 2, i % 2))))

            # ---- main loop with budget packing
            uidx = 0
            for w in range(NW):
                for c in range(SB):
                    g = 16 * w + c
                    emit_qk_exp(w, c)
                    for fn in sched.get(g, ()):
                        fn()
                    budget = 1038 - 430 - mcost.get(g, 0)
                    spent = 0
                    while uidx < len(units):
                        e, d, cost, fn = units[uidx]
                        if e > g:
                            break
                        if (d <= g or spent + cost <= budget + 120
                                or (spent == 0 and budget >= 300)):
                            fn()
                            spent += cost
                            uidx += 1
                        else:
                            break

            # ---- tail: leftover units, then finish window 7.  onmT copies
            # (DVE) go ahead of the outproj staging copies; staging copies
            # alternate ACT/DVE.
            for fn in sched.get(200, ()):
                fn()
            # direct-from-PSUM drain of window 7: normalization runs on ACT
            # (idle after the last exp), per-qc so transposes and outproj
            # pipeline behind it.
            O7 = otiles.pop(7)
            rc7 = rcp.tile([128, 8], F32, tag="rc", name="rc_7")
            nc.vector.reciprocal_approx_fast(out=rc7[:, :], in_=O7[:, 512:520])
            nrm7 = nrmp.tile([128, 512], BF, tag="nrm", name="nrm_7")
            CPY = mybir.ActivationFunctionType.Copy
            for qc in range(4):
                for h in range(2):
                    col = qc * 128 + h * 64
                    if h == 0:
                        nc.scalar.activation(
                            nrm7[:, col:col + 64], O7[:, col:col + 64], CPY,
                            scale=rc7[:, qc * 2 + h: qc * 2 + h + 1])
                    else:
                        nc.vector.tensor_scalar_mul(
                            nrm7[:, col:col + 64], O7[:, col:col + 64],
                            rc7[:, qc * 2 + h: qc * 2 + h + 1])
                tp7 = pjp.tile([128, 128], BF, tag="pj", name=f"tp_7_{qc}")
                nc.tensor.transpose(
                    tp7[:, :], nrm7[:, qc * 128:(qc + 1) * 128], ident[:, :])
                nc.vector.tensor_copy(
                    onmT_sb[:, S + 3 * 512 + qc * 128: S + 3 * 512 +
                            (qc + 1) * 128], tp7[:, :])
                po2 = scp.tile([128, 1024], F32, tag="sc",
                               name=f"po3_{qc}")
                t = 12 + qc
                for n in range(2):
                    for hp in range(2):
                        nc.tensor.matmul(
                            po2[:, n * 512:(n + 1) * 512],
                            lhsT=onmT_sb[:, hp * S + 3 * 512 + qc * 128:
                                         hp * S + 3 * 512 + (qc + 1) * 128],
                            rhs=wo_sb[:, hp * D + n * 512:
                                      hp * D + (n + 1) * 512],
                            start=(hp == 0), stop=(hp == 1),
                        )
                for n in range(2):
                    ot = otp.tile([128, 512], BF, tag="ot",
                                  name=f"ot3_{qc}_{n}")
                    if (qc + n) % 2 == 0:
                        nc.scalar.copy(ot[:, :], po2[:, n * 512:(n + 1) * 512])
                    else:
                        nc.vector.tensor_copy(
                            ot[:, :], po2[:, n * 512:(n + 1) * 512])
                    nc.sync.dma_start(
                        out_d[t * 128:(t + 1) * 128, n * 512:(n + 1) * 512],
                        ot[:, :])
            while uidx < len(units):
                units[uidx][3]()
                uidx += 1
            nc.sync.dma_start(dbg_qt[:, :], qt_sb[:, :])
            nc.sync.dma_start(dbg_kt[:, :], kt_sb[:, :])
            nc.sync.dma_start(dbg_v[:, :], v_sb[:, :])
            nc.sync.dma_start(dbg_onmT[:, :], onmT_sb[:, :])

    nc.compile()
    return nc


def _get_nc(with_bias=False):
    if with_bias not in _nc_cache:
        _nc_cache[with_bias] = _build_bass(with_bias=with_bias)
    return _nc_cache[with_bias]


def _prepare_in_maps(x, wq, bq, wk, bk, wv, bv, wo):
    import ml_dtypes

    bf16 = ml_dtypes.bfloat16
    x = np.asarray(x, np.float32)
    wq, bq = np.asarray(wq, np.float32), np.asarray(bq, np.float32)
    wk, bk = np.asarray(wk, np.float32), np.asarray(bk, np.float32)
    wv, bv = np.asarray(wv, np.float32), np.asarray(bv, np.float32)
    wo = np.asarray(wo, np.float32)

    xT = [np.ascontiguousarray(x[b].T).astype(bf16) for b in range(B)]
    in_maps = []
    for c in range(NC):
        b, j = divmod(c, HPC)
        cs = slice(DHC * j, DHC * (j + 1))
        bias3 = np.concatenate([bq[cs], bk[cs], bv[cs]]).reshape(1, 3 * DHC).astype(bf16)
        in_maps.append(
            {
                "xT": xT[b],
                "wq_c": np.ascontiguousarray(wq[:, cs]).astype(bf16),
                "wk_c": np.ascontiguousarray(wk[:, cs]).astype(bf16),
                "wv_c": np.ascontiguousarray(wv[:, cs]).astype(bf16),
                "wo_c": np.ascontiguousarray(wo[cs, :]).astype(bf16),
                "bias3": np.ascontiguousarray(bias3),
            }
        )
    return in_maps


def _gather(parts, bo):
    bo = np.asarray(bo, np.float32)
    out = np.empty((B, S, D), np.float32)
    for b in range(B):
        acc = np.asarray(parts[HPC * b], np.float32)
        for j in range(1, HPC):
            acc = acc + np.asarray(parts[HPC * b + j], np.float32)
        out[b] = acc + bo
    return out


def kernel(x, wq, bq, wk, bk, wv, bv, wo, bo):
    from concourse import bass_utils

    in_maps = _prepare_in_maps(x, wq, bq, wk, bk, wv, bv, wo)
    with_bias = bool(
        np.any(np.asarray(bq)) or np.any(np.asarray(bk)) or np.any(np.asarray(bv))
    )
    res = bass_utils.run_bass_kernel_spmd(
        nc=_get_nc(with_bias), in_maps=in_maps, core_ids=list(range(NC))
    )
    parts = [np.asarray(r["out"], np.float32) for r in res.results]
    return _gather(parts, bo)


# revision 48
# speedup vs baseline: 1.2354x; 1.0194x over previous
"""Trainium2 Bass kernel for nn_Attention_80384607912675.

Multi-head attention (B=2, S=2048, D=1024, H=16, HD=64), fp32 reference.

Sharding (8 cores): data-parallel over batch (2) x tensor-parallel over heads
(4 head groups of 4 heads).  Core c handles batch c//4, heads [4*(c%4), 4*(c%4)+4).
wq/wk/wv split column-wise, wo split row-wise; the wo partial sums (and the
bias bo) are reduced on the host in fp32.

Per-core kernel (all matmuls bf16, fp32 PSUM accumulation):
  QT/KT = (x @ wq/k + b)^T   head-major [128 (2 heads x 64), 2048] per pair
  V     = x @ wv + bv        natural    [2048, 256] (xT as lhsT -> no transpose)
  per window w = (head pair hp, 512-wide q window qw), kp-chunk c:
    S^T[kp, (h, q)] = K_h^T (x) Q_h      packed [128, 1024] PSUM (A|B)
    P^T             = exp(S^T / 8)       one ACT instr -> bf16 SBUF
    O[q, (qc,h,hd)] += P^T(x)V chunks    [128 q, 64] tiles, full-partition PE
    rs[q, (qc,h)]   += P^T(x)1           rowsums, 1-wide matmuls
  drain: O+rs PSUM -> SBUF copy (frees the single O bank fast), reciprocal,
  per-partition normalize mul, PE transpose to O^T, out = O^T.T @ wo_c
  -> bf16 [2048, 1024] partial, DMA per [128, 512] tile.

Timing model notes (TimelineSim): PE matmul cost = out free size x 0.42ns x
k-chunks, independent of M/K utilization -- so PV uses full 128 q-partitions
(half the cost of the [65, 1024] O^T layout) and rowsums are near-free 1-wide
matmuls.  ACT exp is the second-closest engine to the roofline (~133us busy);
PE is the critical engine (~144us).  The schedule is exp-slot based: per slot
one sc QK pair + one exp, with PV lagged ~12 slots behind (the single PSUM O
bank serializes windows via the drain), and all projection/V/outproj work
packed greedily into per-slot PE budget via a FIFO unit queue with
earliest-slot gates and force-emit deadlines.  CRITICAL INVARIANT: a unit
producing data read at slot g must have deadline <= g-1, because slot g's
QK/exp are emitted first and emission order defines dependency direction in
the tile framework.  Dummy identity transposes burn the PE pstate ramp
(full speed needs ~3us of continuous execution) during the input-DMA wait;
xT streams in two s-halves so lead projections start ~5us in; window 7
drains directly from PSUM with normalize split ACT/DVE to shorten the tail.

Verified: CoreSim core-0 max err 1.7e-3 vs fp64 numpy; full 8-core test
rel err 5.3e-3 (gate 2e-2).  207.1us (baseline) -> 167.6us.
"""

import numpy as np

B, S, D, H = 2, 2048, 1024, 16
HD = D // H          # 64
HPC = 4              # heads per core
DHC = HPC * HD       # 256 head dims per core
KC = D // 128        # 8 contraction chunks
SB = S // 128        # 16 s blocks / kp chunks
NC = 8               # cores
NW = 8               # windows (2 head pairs x 4 q windows)

_nc_cache = {}


def _build_bass(with_bias=False):
    import concourse.mybir as mybir
    import concourse.tile as tile
    from concourse import bacc
    from concourse.masks import make_identity

    BF = mybir.dt.bfloat16
    F32 = mybir.dt.float32
    EXP = mybir.ActivationFunctionType.Exp

    nc = bacc.Bacc("TRN2")

    xT_d = nc.dram_tensor("xT", [D, S], BF, kind="ExternalInput")
    wq_d = nc.dram_tensor("wq_c", [D, DHC], BF, kind="ExternalInput")
    wk_d = nc.dram_tensor("wk_c", [D, DHC], BF, kind="ExternalInput")
    wv_d = nc.dram_tensor("wv_c", [D, DHC], BF, kind="ExternalInput")
    wo_d = nc.dram_tensor("wo_c", [DHC, D], BF, kind="ExternalInput")
    bias_d = nc.dram_tensor("bias3", [1, 3 * DHC], BF, kind="ExternalInput")
    out_d = nc.dram_tensor("out", [S, D], BF, kind="ExternalOutput")

    with tile.TileContext(nc) as tc:
        with (
            tc.tile_pool(name="persist", bufs=1) as pp,
            tc.tile_pool(name="sc", bufs=2, space="PSUM") as scp,
            tc.tile_pool(name="oacc", bufs=1, space="PSUM") as opp,
            tc.tile_pool(name="pj", bufs=2, space="PSUM") as pjp,
            tc.tile_pool(name="pt", bufs=12) as ptp,
            tc.tile_pool(name="osb", bufs=2) as osbp,
            tc.tile_pool(name="nrm", bufs=2) as nrmp,
            tc.tile_pool(name="rc", bufs=2) as rcp,
            tc.tile_pool(name="ot", bufs=8) as otp,
        ):
            xT_sb = pp.tile([128, KC * S], BF, tag="xT", name="xT_sb")
            wq_sb = pp.tile([128, KC * DHC], BF, tag="wq", name="wq_sb")
            wk_sb = pp.tile([128, KC * DHC], BF, tag="wk", name="wk_sb")
            wv_sb = pp.tile([128, KC * DHC], BF, tag="wv", name="wv_sb")
            wo_sb = pp.tile([128, 2 * D], BF, tag="wo", name="wo_sb")
            qt_sb = pp.tile([128, 2 * S], BF, tag="qt", name="qt_sb")
            kt_sb = pp.tile([128, 2 * S], BF, tag="kt", name="kt_sb")
            v_sb = pp.tile([128, SB * DHC], BF, tag="v", name="v_sb")
            onmT_sb = pp.tile([128, 2 * S], BF, tag="onmT", name="onmT_sb")
            ident = pp.tile([128, 128], BF, tag="ident", name="ident")
            bias_sb = pp.tile([1, 3 * DHC], BF, tag="bias", name="bias_sb")
            ones16 = pp.tile([1, 512], BF, tag="ones16", name="ones16")
            ones_col = pp.tile([128, 1], BF, tag="ones_col", name="ones_col")

            # ---- input DMAs: small weights first, xT streamed in s-halves
            # (16 DMAs) so nt0/nt1-dependent projections start early; wo last.
            def load_w(w_sb, w_d):
                nc.sync.dma_start(
                    w_sb[:, :].rearrange("p (k d) -> p k d", d=DHC),
                    w_d[:, :].rearrange("(k p) d -> p k d", p=128),
                )

            def load_xt(k, h):
                nc.sync.dma_start(
                    xT_sb[:, k * S + h * 1024: k * S + (h + 1) * 1024],
                    xT_d[k * 128:(k + 1) * 128, h * 1024:(h + 1) * 1024],
                )

            load_w(wk_sb, wk_d)
            load_xt(0, 0)
            load_xt(1, 0)
            load_w(wq_sb, wq_d)
            for k in range(2, KC):
                load_xt(k, 0)
            load_w(wv_sb, wv_d)
            for k in range(KC):
                load_xt(k, 1)
            nc.sync.dma_start(bias_sb[:, :], bias_d[:, :])
            nc.sync.dma_start(
                wo_sb[:, :].rearrange("r (p d) -> r p d", d=D),
                wo_d[:, :].rearrange("(p r) d -> r p d", r=128),
            )
            nc.vector.memset(ones16[:, :], 1.0)
            nc.vector.memset(ones_col[:, :], 1.0)
            make_identity(nc, ident[:, :])

            bq = bias_sb[0:1, 0:DHC]
            bk = bias_sb[0:1, DHC:2 * DHC]
            bv = bias_sb[0:1, 2 * DHC:3 * DHC]

            # ---- Q/K projection tiles (p: head-pair block, nt: 512 s cols),
            # emitted in four 2-k-chunk quarters so no single filler slot
            # exceeds the exp budget (locally PE-stalled exp slots are never
            # recovered).
            pend = {}

            def qk_quarter(dst_sb, w_sb, bias, p, nt, q, eng="dve"):
                key = (dst_sb.tensor.name, p, nt)
                if q == 0:
                    ps = pjp.tile([128, 512], F32, tag="pj",
                                  name=f"qk_{key[0]}_{p}_{nt}")
                    pend[key] = ps
                else:
                    ps = pend[key]
                for k in (2 * q, 2 * q + 1):
                    nc.tensor.matmul(
                        ps[:, :],
                        lhsT=w_sb[:, k * DHC + p * 128: k * DHC + (p + 1) * 128],
                        rhs=xT_sb[:, k * S + nt * 512: k * S + (nt + 1) * 512],
                        start=(k == 0),
                        stop=(k == KC - 1 and not with_bias),
                    )
                if q == 3:
                    del pend[key]
                    if with_bias:
                        nc.tensor.matmul(
                            ps[:, :],
                            lhsT=bias[:, p * 128:(p + 1) * 128],
                            rhs=ones16[0:1, :],
                            start=False, stop=True,
                        )
                    dst = dst_sb[:, p * S + nt * 512: p * S + (nt + 1) * 512]
                    if eng == "act":
                        nc.scalar.copy(dst, ps[:, :])
                    else:
                        nc.vector.tensor_copy(dst, ps[:, :])

            # ---- V pair tiles: pair j covers s-chunks 2j, 2j+1 in natural
            # layout (lhsT = xT s-slice, rhs = wv chunk), in 4-matmul
            # quarters.  Pairs 0/1 run in the lead psum slots (lv).
            vpend = {}

            def v_bias_mms(ps):
                for h2 in range(2):
                    nc.tensor.matmul(
                        ps[:, h2 * 256:(h2 + 1) * 256],
                        lhsT=ones16[0:1, 0:128], rhs=bv[:, :],
                        start=False, stop=(h2 == 1),
                    )

            def v_quarter(j, q):
                if j < 2:
                    ps = lv[j]
                elif q == 0:
                    ps = pjp.tile([128, 512], F32, tag="pj", name=f"v_{j}")
                    vpend[j] = ps
                else:
                    ps = vpend[j]
                half = q // 2
                sc_ = 2 * j + half
                for k in range(4 * (q % 2), 4 * (q % 2) + 4):
                    nc.tensor.matmul(
                        ps[:, half * 256:(half + 1) * 256],
                        lhsT=xT_sb[:, k * S + sc_ * 128: k * S + (sc_ + 1) * 128],
                        rhs=wv_sb[:, k * DHC: k * DHC + DHC],
                        start=(k == 0 and half == 0),
                        stop=(k == KC - 1 and half == 1 and not with_bias),
                    )
                if q == 3:
                    vpend.pop(j, None)
                    if with_bias:
                        v_bias_mms(ps)
                    nc.vector.tensor_copy(
                        v_sb[:, 2 * j * DHC: (2 * j + 2) * DHC],
                        ps[:, 0:512])

            # ---- per-window attention pieces
            otiles = {}
            pts = {}

            def emit_qk_exp(w, c):
                hp, qw = w // 4, w % 4
                sc = scp.tile([128, 1024], F32, tag="sc", name=f"sc_{w}_{c}")
                for i in range(2):
                    nc.tensor.matmul(
                        sc[:, 512 * i:512 * (i + 1)],
                        lhsT=kt_sb[64 * i:64 * (i + 1),
                                   hp * S + c * 128: hp * S + (c + 1) * 128],
                        rhs=qt_sb[64 * i:64 * (i + 1),
                                  hp * S + qw * 512: hp * S + (qw + 1) * 512],
                        start=True, stop=True,
                    )
                pt = ptp.tile([128, 1024], BF, tag="pt", name=f"pt_{w}_{c}")
                nc.scalar.activation(pt[:, :], sc[:, :], EXP, scale=0.125)
                pts[(w, c)] = pt

            def pv(w, c):
                hp = w // 4
                if c == 0:
                    otiles[w] = opp.tile([128, 520], F32, tag="oacc",
                                         name=f"o_{w}")
                O = otiles[w]
                pt = pts.pop((w, c))
                for qc in range(4):
                    for h in range(2):
                        first = (c == 0 and qc == 0 and h == 0)
                        last = (c == SB - 1 and qc == 3 and h == 1)
                        lh = pt[:, h * 512 + qc * 128: h * 512 + (qc + 1) * 128]
                        nc.tensor.matmul(
                            O[:, qc * 128 + h * 64: qc * 128 + h * 64 + 64],
                            lhsT=lh,
                            rhs=v_sb[:, c * DHC + (2 * hp + h) * 64:
                                     c * DHC + (2 * hp + h) * 64 + 64],
                            start=first, stop=last,
                        )
                        nc.tensor.matmul(
                            O[:, 512 + qc * 2 + h: 513 + qc * 2 + h],
                            lhsT=lh, rhs=ones_col[:, 0:1],
                            start=first, stop=last,
                        )

            osbs = {}
            nrms = {}

            def drain_a(w):
                osb = osbp.tile([128, 520], F32, tag="osb", name=f"osb_{w}")
                nc.vector.tensor_copy(osb[:, :], otiles.pop(w)[:, :])
                osbs[w] = osb

            def drain_b(w):
                osb = osbs.pop(w)
                rc = rcp.tile([128, 8], F32, tag="rc", name=f"rc_{w}")
                nc.vector.reciprocal_approx_fast(
                    out=rc[:, :], in_=osb[:, 512:520])
                nrm = nrmp.tile([128, 512], BF, tag="nrm", name=f"nrm_{w}")
                for qc in range(4):
                    for h in range(2):
                        col = qc * 128 + h * 64
                        nc.vector.tensor_scalar_mul(
                            nrm[:, col:col + 64], osb[:, col:col + 64],
                            rc[:, qc * 2 + h: qc * 2 + h + 1])
                nrms[w] = nrm

            def drain_tp(w, qc):
                hp, qw = w // 4, w % 4
                nrm = nrms[w]
                tp = pjp.tile([128, 128], BF, tag="pj", name=f"tp_{w}_{qc}")
                nc.tensor.transpose(
                    tp[:, :], nrm[:, qc * 128:(qc + 1) * 128], ident[:, :])
                nc.vector.tensor_copy(
                    onmT_sb[:, hp * S + qw * 512 + qc * 128:
                            hp * S + qw * 512 + (qc + 1) * 128], tp[:, :])
                if qc == 3:
                    del nrms[w]

            def outproj(qw, qc, n, eng="dve"):
                t = qw * 4 + qc
                po = pjp.tile([128, 512], F32, tag="pj", name=f"po_{t}_{n}")
                for hp in range(2):
                    nc.tensor.matmul(
                        po[:, :],
                        lhsT=onmT_sb[:, hp * S + qw * 512 + qc * 128:
                                     hp * S + qw * 512 + (qc + 1) * 128],
                        rhs=wo_sb[:, hp * D + n * 512: hp * D + (n + 1) * 512],
                        start=(hp == 0), stop=(hp == 1),
                    )
                ot = otp.tile([128, 512], BF, tag="ot", name=f"ot_{t}_{n}")
                if eng == "act":
                    nc.scalar.copy(ot[:, :], po[:, :])
                else:
                    nc.vector.tensor_copy(ot[:, :], po[:, :])
                nc.sync.dma_start(
                    out_d[t * 128:(t + 1) * 128, n * 512:(n + 1) * 512],
                    ot[:, :])

            # ---- lead-in: kt/qt (p0, nt0) + V pairs 0,1 pipelined against
            # the arriving xT halves; kt/qt finish first so window 0 starts
            # as early as possible.  Dummy identity transposes keep the PE
            # continuously busy from t~0 so the pstate ramp (full speed after
            # 3us of uninterrupted execution) is burned during the input DMA
            # instead of doubling every lead matmul.
            def ramp(n):
                for _ in range(n):
                    nc.tensor.transpose(
                        dummy_bf[:, :], ident[:, :], ident[:, :])

            dummy_bf = pjp.tile([128, 128], BF, tag="pj", name="dummy_bf")
            lt = scp.tile([128, 1024], F32, tag="sc", name="lead_ktqt")
            lv = [opp.tile([128, 520], F32, tag="oacc", name="lead_v01"),
                  pjp.tile([128, 512], F32, tag="pj", name="lead_v23")]

            def lead_mm(k):
                for half, (w_sb,) in enumerate([(wk_sb,), (wq_sb,)]):
                    nc.tensor.matmul(
                        lt[:, half * 512:(half + 1) * 512],
                        lhsT=w_sb[:, k * DHC: k * DHC + 128],
                        rhs=xT_sb[:, k * S: k * S + 512],
                        start=(k == 0),
                        stop=(k == KC - 1 and not with_bias),
                    )

            def lead_vmm(k, pair):
                ps = lv[pair]
                for h2 in range(2):
                    sc_ = 2 * pair + h2
                    nc.tensor.matmul(
                        ps[:, h2 * 256:(h2 + 1) * 256],
                        lhsT=xT_sb[:, k * S + sc_ * 128: k * S + (sc_ + 1) * 128],
                        rhs=wv_sb[:, k * DHC: k * DHC + DHC],
                        start=(k == 0 and h2 == 0),
                        stop=(k == KC - 1 and h2 == 1 and not with_bias),
                    )

            ramp(15)
            for k in range(KC):
                lead_mm(k)
                if k < KC - 1:
                    ramp(2 if k < 4 else 5)
            if with_bias:
                for half, bias in enumerate([bk, bq]):
                    nc.tensor.matmul(
                        lt[:, half * 512:(half + 1) * 512],
                        lhsT=bias[:, 0:128], rhs=ones16[0:1, :],
                        start=False, stop=True,
                    )
            nc.scalar.copy(kt_sb[:, 0:512], lt[:, 0:512])
            nc.vector.tensor_copy(qt_sb[:, 0:512], lt[:, 512:1024])

            # ---- schedule ----
            # Mandatory per-slot items (PV cadence, drains, transposes) are
            # placed at fixed global slots g = 16*w + c.  Everything else
            # (projection/V quarters, outproj pieces) lives in a strict-FIFO
            # unit queue packed greedily against each slot's leftover PE
            # budget, with earliest-slot gates and force-emit deadlines.
            sched = {}
            mcost = {}

            def at(g, fn, cost=0):
                sched.setdefault(g, []).append(fn)
                mcost[g] = mcost.get(g, 0) + cost

            # PV cadence: PV(w, c) lags exp by ~12 slots; the single O bank
            # serializes windows (drain_a frees it).  Last two windows
            # compress so the tail stays short.
            for w in range(NW):
                for c in range(SB):
                    if w < 6:
                        g = 16 * w + 12 + c
                    elif w == 6:
                        g = 108 + c if c < 8 else 112 + (c - 8) // 2
                    else:
                        g = (120 + c if c < 4 else
                             124 + (c - 4) // 2 if c < 12 else 200)
                    at(g, lambda w=w, c=c: pv(w, c), 240)
            for w in range(NW - 1):
                if w < 6:
                    ga, gtp = 16 * w + 27, 16 * w + 30
                else:
                    ga, gtp = 115, 118
                at(ga, lambda w=w: drain_a(w))
                at(ga + 1, lambda w=w: drain_b(w))
                for qc in range(4):
                    at(gtp + qc // 2, lambda w=w, qc=qc: drain_tp(w, qc), 60)

            # filler unit queue
            units = []

            def unit(e, dls, cost, mk):
                for q, d in enumerate(dls):
                    units.append((e[q] if isinstance(e, list) else e, d, cost,
                                  mk(q)))

            def mk_qk(dst, wt, bias, p, nt, eng="dve"):
                return lambda q: (lambda: qk_quarter(dst, wt, bias, p, nt, q,
                                                     eng))

            def mk_v(j):
                return lambda q: (lambda: v_quarter(j, q))

            def mk_po(qw, qc, n, eng="dve"):
                return lambda q: (lambda: outproj(qw, qc, n, eng))

            unit(0, [0, 1, 2, 3], 430, mk_qk(kt_sb, wk_sb, bk, 0, 1, "act"))
            unit(1, [3, 4, 5, 6], 430, mk_v(0))
            unit([1, 2, 3, 5], [4, 5, 6, 7], 430,
                 mk_qk(kt_sb, wk_sb, bk, 0, 2, "act"))
            unit(1, [5, 6, 7, 8], 430, mk_v(1))
            unit(0, [8, 9, 10, 11], 430, mk_v(2))
            unit([1, 2, 3, 5], [8, 9, 10, 11], 430,
                 mk_qk(kt_sb, wk_sb, bk, 0, 3, "act"))
            unit(0, [10, 11, 12, 13], 430, mk_v(3))
            unit(0, [12, 13, 14, 15], 430, mk_qk(qt_sb, wq_sb, bq, 0, 1, "act"))
            unit([3, 5, 5, 5], [14, 15, 16, 17], 430, mk_v(4))
            unit([4, 5, 5, 6], [18, 19, 20, 21], 430, mk_v(5))
            unit([4, 5, 6, 6], [20, 21, 22, 23], 430, mk_v(6))
            unit([5, 6, 6, 7], [22, 23, 24, 25], 430, mk_v(7))
            unit(0, [28, 29, 30, 31], 430, mk_qk(qt_sb, wq_sb, bq, 0, 2))
            unit(0, [44, 45, 46, 47], 430, mk_qk(qt_sb, wq_sb, bq, 0, 3))
            unit(0, [60, 61, 62, 63], 430, mk_qk(kt_sb, wk_sb, bk, 1, 0))
            unit(0, [64, 65, 66, 67], 430, mk_qk(kt_sb, wk_sb, bk, 1, 1))
            unit(0, [68, 69, 70, 71], 430, mk_qk(kt_sb, wk_sb, bk, 1, 2))
            unit(0, [72, 73, 74, 75], 430, mk_qk(kt_sb, wk_sb, bk, 1, 3))
            unit(0, [60, 61, 62, 63], 430, mk_qk(qt_sb, wq_sb, bq, 1, 0))
            unit(0, [76, 77, 78, 79], 430, mk_qk(qt_sb, wq_sb, bq, 1, 1))
            unit([88, 89, 90, 91], [92, 93, 94, 95], 430, mk_qk(qt_sb, wq_sb, bq, 1, 2))
            for i in range(8):
                units.append((96, 118 + i, 430, (lambda i=i: outproj(
                    0, i // 2, i % 2))))
            unit([100, 101, 102, 103], [108, 109, 110, 111], 430, mk_qk(qt_sb, wq_sb, bq, 1, 3))
            for i in range(8):
                units.append((112, 113 + i, 430, (lambda i=i: outproj(
                    1, i // 2, i % 2))))
            for i in range(8):
                units.append((120, 120 + i, 430, (lambda i=i: outproj(
                    2, i // 2, i % 2))))

            # ---- main loop with budget packing
            uidx = 0
            for w in range(NW):
                for c in range(SB):
                    g = 16 * w + c
                    emit_qk_exp(w, c)
                    for fn in sched.get(g, ()):
                        fn()
                    budget = 1038 - 430 - mcost.get(g, 0)
                    spent = 0
                    while uidx < len(units):
                        e, d, cost, fn = units[uidx]
                        if e > g:
                            break
                        if (d <= g or spent + cost <= budget + 120
                                or (spent == 0 and budget >= 300)):
                            fn()
                            spent += cost
                            uidx += 1
                        else:
                            break

            # ---- tail: leftover units, then finish window 7.  onmT copies
            # (DVE) go ahead of the outproj staging copies; staging copies
            # alternate ACT/DVE.
            for fn in sched.get(200, ()):
                fn()
            # direct-from-PSUM drain of window 7: normalization runs on ACT
            # (idle after the last exp), per-qc so transposes and outproj
            # pipeline behind it.
            O7 = otiles.pop(7)
            rc7 = rcp.tile([128, 8], F32, tag="rc", name="rc_7")
            nc.vector.reciprocal_approx_fast(out=rc7[:, :], in_=O7[:, 512:520])
            nrm7 = nrmp.tile([128, 512], BF, tag="nrm", name="nrm_7")
            CPY = mybir.ActivationFunctionType.Copy
            for qc in range(4):
                for h in range(2):
                    col = qc * 128 + h * 64
                    if h == 0:
                        nc.scalar.activation(
                            nrm7[:, col:col + 64], O7[:, col:col + 64], CPY,
                            scale=rc7[:, qc * 2 + h: qc * 2 + h + 1])
                    else:
                        nc.vector.tensor_scalar_mul(
                            nrm7[:, col:col + 64], O7[:, col:col + 64],
                            rc7[:, qc * 2 + h: qc * 2 + h + 1])
                tp7 = pjp.tile([128, 128], BF, tag="pj", name=f"tp_7_{qc}")
                nc.tensor.transpose(
                    tp7[:, :], nrm7[:, qc * 128:(qc + 1) * 128], ident[:, :])
                nc.vector.tensor_copy(
                    onmT_sb[:, S + 3 * 512 + qc * 128: S + 3 * 512 +
                            (qc + 1) * 128], tp7[:, :])
                po2 = scp.tile([128, 1024], F32, tag="sc",
                               name=f"po3_{qc}")
                t = 12 + qc
                for n in range(2):
                    for hp in range(2):
                        nc.tensor.matmul(
                            po2[:, n * 512:(n + 1) * 512],
                            lhsT=onmT_sb[:, hp * S + 3 * 512 + qc * 128:
                                         hp * S + 3 * 512 + (qc + 1) * 128],
                            rhs=wo_sb[:, hp * D + n * 512:
                                      hp * D + (n + 1) * 512],
                            start=(hp == 0), stop=(hp == 1),
                        )
                for n in range(2):
                    ot = otp.tile([128, 512], BF, tag="ot",
                                  name=f"ot3_{qc}_{n}")
                    if (qc + n) % 2 == 0:
                        nc.scalar.copy(ot[:, :], po2[:, n * 512:(n + 1) * 512])
                    else:
                        nc.vector.tensor_copy(
                            ot[:, :], po2[:, n * 512:(n + 1) * 512])
                    nc.sync.dma_start(
                        out_d[t * 128:(t + 1) * 128, n * 512:(n + 1) * 512],
                        ot[:, :])
            while uidx < len(units):
                units[uidx][3]()
                uidx += 1

    nc.compile()
    return nc


def _get_nc(with_bias=False):
    if with_bias not in _nc_cache:
        _nc_cache[with_bias] = _build_bass(with_bias=with_bias)
    return _nc_cache[with_bias]


def _prepare_in_maps(x, wq, bq, wk, bk, wv, bv, wo):
    import ml_dtypes

    bf16 = ml_dtypes.bfloat16
    x = np.asarray(x, np.float32)
    wq, bq = np.asarray(wq, np.float32), np.asarray(bq, np.float32)
    wk, bk = np.asarray(wk, np.float32), np.asarray(bk, np.float32)
    wv, bv = np.asarray(wv, np.float32), np.asarray(bv, np.float32)
    wo = np.asarray(wo, np.float32)

    xT = [np.ascontiguousarray(x[b].T).astype(bf16) for b in range(B)]
    in_maps = []
    for c in range(NC):
        b, j = divmod(c, HPC)
        cs = slice(DHC * j, DHC * (j + 1))
        bias3 = np.concatenate([bq[cs], bk[cs], bv[cs]]).reshape(1, 3 * DHC).astype(bf16)
        in_maps.append(
            {
                "xT": xT[b],
                "wq_c": np.ascontiguousarray(wq[:, cs]).astype(bf16),
                "wk_c": np.ascontiguousarray(wk[:, cs]).astype(bf16),
                "wv_c": np.ascontiguousarray(wv[:, cs]).astype(bf16),
                "wo_c": np.ascontiguousarray(wo[cs, :]).astype(bf16),
                "bias3": np.ascontiguousarray(bias3),
            }
        )
    return in_maps


def _gather(parts, bo):
    bo = np.asarray(bo, np.float32)
    out = np.empty((B, S, D), np.float32)
    for b in range(B):
        acc = np.asarray(parts[HPC * b], np.float32)
        for j in range(1, HPC):
            acc = acc + np.asarray(parts[HPC * b + j], np.float32)
        out[b] = acc + bo
    return out


def kernel(x, wq, bq, wk, bk, wv, bv, wo, bo):
    from concourse import bass_utils

    in_maps = _prepare_in_maps(x, wq, bq, wk, bk, wv, bv, wo)
    with_bias = bool(
        np.any(np.asarray(bq)) or np.any(np.asarray(bk)) or np.any(np.asarray(bv))
    )
    res = bass_utils.run_bass_kernel_spmd(
        nc=_get_nc(with_bias), in_maps=in_maps, core_ids=list(range(NC))
    )
    parts = [np.asarray(r["out"], np.float32) for r in res.results]
    return _gather(parts, bo)


# revision 49
# speedup vs baseline: 1.2358x; 1.0003x over previous
"""Trainium2 Bass kernel for nn_Attention_80384607912675.

Multi-head attention (B=2, S=2048, D=1024, H=16, HD=64), fp32 reference.

Sharding (8 cores): data-parallel over batch (2) x tensor-parallel over heads
(4 head groups of 4 heads).  Core c handles batch c//4, heads [4*(c%4), 4*(c%4)+4).
wq/wk/wv split column-wise, wo split row-wise; the wo partial sums (and the
bias bo) are reduced on the host in fp32.

Per-core kernel (all matmuls bf16, fp32 PSUM accumulation):
  QT/KT = (x @ wq/k + b)^T   head-major [128 (2 heads x 64), 2048] per pair
  V     = x @ wv + bv        natural    [2048, 256] (xT as lhsT -> no transpose)
  per window w = (head pair hp, 512-wide q window qw), kp-chunk c:
    S^T[kp, (h, q)] = K_h^T (x) Q_h      packed [128, 1024] PSUM (A|B)
    P^T             = exp(S^T / 8)       one ACT instr -> bf16 SBUF
    O[q, (qc,h,hd)] += P^T(x)V chunks    [128 q, 64] tiles, full-partition PE
    rs[q, (qc,h)]   += P^T(x)1           rowsums, 1-wide matmuls
  drain: O+rs PSUM -> SBUF copy (frees the single O bank fast), reciprocal,
  per-partition normalize mul, PE transpose to O^T, out = O^T.T @ wo_c
  -> bf16 [2048, 1024] partial, DMA per [128, 512] tile.

Timing model notes (TimelineSim): PE matmul cost = out free size x 0.42ns x
k-chunks, independent of M/K utilization -- so PV uses full 128 q-partitions
(half the cost of the [65, 1024] O^T layout) and rowsums are near-free 1-wide
matmuls.  ACT exp is the second-closest engine to the roofline (~133us busy);
PE is the critical engine (~144us).  The schedule is exp-slot based: per slot
one sc QK pair + one exp, with PV lagged ~12 slots behind (the single PSUM O
bank serializes windows via the drain), and all projection/V/outproj work
packed greedily into per-slot PE budget via a FIFO unit queue with
earliest-slot gates and force-emit deadlines.  CRITICAL INVARIANT: a unit
producing data read at slot g must have deadline <= g-1, because slot g's
QK/exp are emitted first and emission order defines dependency direction in
the tile framework.  Dummy identity transposes burn the PE pstate ramp
(full speed needs ~3us of continuous execution) during the input-DMA wait;
xT streams in two s-halves so lead projections start ~5us in; window 7
drains directly from PSUM with normalize split ACT/DVE to shorten the tail.

Verified: CoreSim core-0 max err 1.7e-3 vs fp64 numpy; full 8-core test
rel err 5.3e-3 (gate 2e-2).  207.1us (baseline) -> 167.6us.
"""

import numpy as np

B, S, D, H = 2, 2048, 1024, 16
HD = D // H          # 64
HPC = 4              # heads per core
DHC = HPC * HD       # 256 head dims per core
KC = D // 128        # 8 contraction chunks
SB = S // 128        # 16 s blocks / kp chunks
NC = 8               # cores
NW = 8               # windows (2 head pairs x 4 q windows)

_nc_cache = {}


def _build_bass(with_bias=False):
    import concourse.mybir as mybir
    import concourse.tile as tile
    from concourse import bacc
    from concourse.masks import make_identity

    BF = mybir.dt.bfloat16
    F32 = mybir.dt.float32
    EXP = mybir.ActivationFunctionType.Exp

    nc = bacc.Bacc("TRN2")

    xT_d = nc.dram_tensor("xT", [D, S], BF, kind="ExternalInput")
    wq_d = nc.dram_tensor("wq_c", [D, DHC], BF, kind="ExternalInput")
    wk_d = nc.dram_tensor("wk_c", [D, DHC], BF, kind="ExternalInput")
    wv_d = nc.dram_tensor("wv_c", [D, DHC], BF, kind="ExternalInput")
    wo_d = nc.dram_tensor("wo_c", [DHC, D], BF, kind="ExternalInput")
    bias_d = nc.dram_tensor("bias3", [1, 3 * DHC], BF, kind="ExternalInput")
    out_d = nc.dram_tensor("out", [S, D], BF, kind="ExternalOutput")

    with tile.TileContext(nc) as tc:
        with (
            tc.tile_pool(name="persist", bufs=1) as pp,
            tc.tile_pool(name="sc", bufs=2, space="PSUM") as scp,
            tc.tile_pool(name="oacc", bufs=1, space="PSUM") as opp,
            tc.tile_pool(name="pj", bufs=2, space="PSUM") as pjp,
            tc.tile_pool(name="pt", bufs=12) as ptp,
            tc.tile_pool(name="osb", bufs=2) as osbp,
            tc.tile_pool(name="nrm", bufs=2) as nrmp,
            tc.tile_pool(name="rc", bufs=2) as rcp,
            tc.tile_pool(name="ot", bufs=8) as otp,
        ):
            xT_sb = pp.tile([128, KC * S], BF, tag="xT", name="xT_sb")
            wq_sb = pp.tile([128, KC * DHC], BF, tag="wq", name="wq_sb")
            wk_sb = pp.tile([128, KC * DHC], BF, tag="wk", name="wk_sb")
            wv_sb = pp.tile([128, KC * DHC], BF, tag="wv", name="wv_sb")
            wo_sb = pp.tile([128, 2 * D], BF, tag="wo", name="wo_sb")
            qt_sb = pp.tile([128, 2 * S], BF, tag="qt", name="qt_sb")
            kt_sb = pp.tile([128, 2 * S], BF, tag="kt", name="kt_sb")
            v_sb = pp.tile([128, SB * DHC], BF, tag="v", name="v_sb")
            onmT_sb = pp.tile([128, 2 * S], BF, tag="onmT", name="onmT_sb")
            ident = pp.tile([128, 128], BF, tag="ident", name="ident")
            bias_sb = pp.tile([1, 3 * DHC], BF, tag="bias", name="bias_sb")
            ones16 = pp.tile([1, 512], BF, tag="ones16", name="ones16")
            ones_col = pp.tile([128, 1], BF, tag="ones_col", name="ones_col")

            # ---- input DMAs: small weights first, xT streamed in s-halves
            # (16 DMAs) so nt0/nt1-dependent projections start early; wo last.
            def load_w(w_sb, w_d):
                nc.sync.dma_start(
                    w_sb[:, :].rearrange("p (k d) -> p k d", d=DHC),
                    w_d[:, :].rearrange("(k p) d -> p k d", p=128),
                )

            def load_xt(k, h):
                nc.sync.dma_start(
                    xT_sb[:, k * S + h * 1024: k * S + (h + 1) * 1024],
                    xT_d[k * 128:(k + 1) * 128, h * 1024:(h + 1) * 1024],
                )

            load_w(wk_sb, wk_d)
            load_xt(0, 0)
            load_xt(1, 0)
            load_w(wq_sb, wq_d)
            for k in range(2, KC):
                load_xt(k, 0)
            load_w(wv_sb, wv_d)
            for k in range(KC):
                load_xt(k, 1)
            nc.sync.dma_start(bias_sb[:, :], bias_d[:, :])
            nc.sync.dma_start(
                wo_sb[:, :].rearrange("r (p d) -> r p d", d=D),
                wo_d[:, :].rearrange("(p r) d -> r p d", r=128),
            )
            nc.vector.memset(ones16[:, :], 1.0)
            nc.vector.memset(ones_col[:, :], 1.0)
            make_identity(nc, ident[:, :])

            bq = bias_sb[0:1, 0:DHC]
            bk = bias_sb[0:1, DHC:2 * DHC]
            bv = bias_sb[0:1, 2 * DHC:3 * DHC]

            # ---- Q/K projection tiles (p: head-pair block, nt: 512 s cols),
            # emitted in four 2-k-chunk quarters so no single filler slot
            # exceeds the exp budget (locally PE-stalled exp slots are never
            # recovered).
            pend = {}

            def qk_quarter(dst_sb, w_sb, bias, p, nt, q, eng="dve"):
                key = (dst_sb.tensor.name, p, nt)
                if q == 0:
                    ps = pjp.tile([128, 512], F32, tag="pj",
                                  name=f"qk_{key[0]}_{p}_{nt}")
                    pend[key] = ps
                else:
                    ps = pend[key]
                for k in (2 * q, 2 * q + 1):
                    nc.tensor.matmul(
                        ps[:, :],
                        lhsT=w_sb[:, k * DHC + p * 128: k * DHC + (p + 1) * 128],
                        rhs=xT_sb[:, k * S + nt * 512: k * S + (nt + 1) * 512],
                        start=(k == 0),
                        stop=(k == KC - 1 and not with_bias),
                    )
                if q == 3:
                    del pend[key]
                    if with_bias:
                        nc.tensor.matmul(
                            ps[:, :],
                            lhsT=bias[:, p * 128:(p + 1) * 128],
                            rhs=ones16[0:1, :],
                            start=False, stop=True,
                        )
                    dst = dst_sb[:, p * S + nt * 512: p * S + (nt + 1) * 512]
                    if eng == "act":
                        nc.scalar.copy(dst, ps[:, :])
                    else:
                        nc.vector.tensor_copy(dst, ps[:, :])

            # ---- V pair tiles: pair j covers s-chunks 2j, 2j+1 in natural
            # layout (lhsT = xT s-slice, rhs = wv chunk), in 4-matmul
            # quarters.  Pairs 0/1 run in the lead psum slots (lv).
            vpend = {}

            def v_bias_mms(ps):
                for h2 in range(2):
                    nc.tensor.matmul(
                        ps[:, h2 * 256:(h2 + 1) * 256],
                        lhsT=ones16[0:1, 0:128], rhs=bv[:, :],
                        start=False, stop=(h2 == 1),
                    )

            def v_quarter(j, q):
                if j < 2:
                    ps = lv[j]
                elif q == 0:
                    ps = pjp.tile([128, 512], F32, tag="pj", name=f"v_{j}")
                    vpend[j] = ps
                else:
                    ps = vpend[j]
                half = q // 2
                sc_ = 2 * j + half
                for k in range(4 * (q % 2), 4 * (q % 2) + 4):
                    nc.tensor.matmul(
                        ps[:, half * 256:(half + 1) * 256],
                        lhsT=xT_sb[:, k * S + sc_ * 128: k * S + (sc_ + 1) * 128],
                        rhs=wv_sb[:, k * DHC: k * DHC + DHC],
                        start=(k == 0 and half == 0),
                        stop=(k == KC - 1 and half == 1 and not with_bias),
                    )
                if q == 3:
                    vpend.pop(j, None)
                    if with_bias:
                        v_bias_mms(ps)
                    nc.vector.tensor_copy(
                        v_sb[:, 2 * j * DHC: (2 * j + 2) * DHC],
                        ps[:, 0:512])

            # ---- per-window attention pieces
            otiles = {}
            pts = {}

            def emit_qk_exp(w, c):
                hp, qw = w // 4, w % 4
                sc = scp.tile([128, 1024], F32, tag="sc", name=f"sc_{w}_{c}")
                for i in range(2):
                    nc.tensor.matmul(
                        sc[:, 512 * i:512 * (i + 1)],
                        lhsT=kt_sb[64 * i:64 * (i + 1),
                                   hp * S + c * 128: hp * S + (c + 1) * 128],
                        rhs=qt_sb[64 * i:64 * (i + 1),
                                  hp * S + qw * 512: hp * S + (qw + 1) * 512],
                        start=True, stop=True,
                    )
                pt = ptp.tile([128, 1024], BF, tag="pt", name=f"pt_{w}_{c}")
                nc.scalar.activation(pt[:, :], sc[:, :], EXP, scale=0.125)
                pts[(w, c)] = pt

            def pv(w, c):
                hp = w // 4
                if c == 0:
                    otiles[w] = opp.tile([128, 520], F32, tag="oacc",
                                         name=f"o_{w}")
                O = otiles[w]
                pt = pts.pop((w, c))
                for qc in range(4):
                    for h in range(2):
                        first = (c == 0 and qc == 0 and h == 0)
                        last = (c == SB - 1 and qc == 3 and h == 1)
                        lh = pt[:, h * 512 + qc * 128: h * 512 + (qc + 1) * 128]
                        nc.tensor.matmul(
                            O[:, qc * 128 + h * 64: qc * 128 + h * 64 + 64],
                            lhsT=lh,
                            rhs=v_sb[:, c * DHC + (2 * hp + h) * 64:
                                     c * DHC + (2 * hp + h) * 64 + 64],
                            start=first, stop=last,
                        )
                        nc.tensor.matmul(
                            O[:, 512 + qc * 2 + h: 513 + qc * 2 + h],
                            lhsT=lh, rhs=ones_col[:, 0:1],
                            start=first, stop=last,
                        )

            osbs = {}
            nrms = {}

            def drain_a(w):
                osb = osbp.tile([128, 520], F32, tag="osb", name=f"osb_{w}")
                nc.vector.tensor_copy(osb[:, :], otiles.pop(w)[:, :])
                osbs[w] = osb

            def drain_b(w):
                osb = osbs.pop(w)
                rc = rcp.tile([128, 8], F32, tag="rc", name=f"rc_{w}")
                nc.vector.reciprocal_approx_fast(
                    out=rc[:, :], in_=osb[:, 512:520])
                nrm = nrmp.tile([128, 512], BF, tag="nrm", name=f"nrm_{w}")
                for qc in range(4):
                    for h in range(2):
                        col = qc * 128 + h * 64
                        nc.vector.tensor_scalar_mul(
                            nrm[:, col:col + 64], osb[:, col:col + 64],
                            rc[:, qc * 2 + h: qc * 2 + h + 1])
                nrms[w] = nrm

            def drain_tp(w, qc):
                hp, qw = w // 4, w % 4
                nrm = nrms[w]
                tp = pjp.tile([128, 128], BF, tag="pj", name=f"tp_{w}_{qc}")
                nc.tensor.transpose(
                    tp[:, :], nrm[:, qc * 128:(qc + 1) * 128], ident[:, :])
                nc.vector.tensor_copy(
                    onmT_sb[:, hp * S + qw * 512 + qc * 128:
                            hp * S + qw * 512 + (qc + 1) * 128], tp[:, :])
                if qc == 3:
                    del nrms[w]

            def outproj(qw, qc, n, eng="dve"):
                t = qw * 4 + qc
                po = pjp.tile([128, 512], F32, tag="pj", name=f"po_{t}_{n}")
                for hp in range(2):
                    nc.tensor.matmul(
                        po[:, :],
                        lhsT=onmT_sb[:, hp * S + qw * 512 + qc * 128:
                                     hp * S + qw * 512 + (qc + 1) * 128],
                        rhs=wo_sb[:, hp * D + n * 512: hp * D + (n + 1) * 512],
                        start=(hp == 0), stop=(hp == 1),
                    )
                ot = otp.tile([128, 512], BF, tag="ot", name=f"ot_{t}_{n}")
                if eng == "act":
                    nc.scalar.copy(ot[:, :], po[:, :])
                else:
                    nc.vector.tensor_copy(ot[:, :], po[:, :])
                nc.sync.dma_start(
                    out_d[t * 128:(t + 1) * 128, n * 512:(n + 1) * 512],
                    ot[:, :])

            # ---- lead-in: kt/qt (p0, nt0) + V pairs 0,1 pipelined against
            # the arriving xT halves; kt/qt finish first so window 0 starts
            # as early as possible.  Dummy identity transposes keep the PE
            # continuously busy from t~0 so the pstate ramp (full speed after
            # 3us of uninterrupted execution) is burned during the input DMA
            # instead of doubling every lead matmul.
            def ramp(n):
                for _ in range(n):
                    nc.tensor.transpose(
                        dummy_bf[:, :], ident[:, :], ident[:, :])

            dummy_bf = pjp.tile([128, 128], BF, tag="pj", name="dummy_bf")
            lt = scp.tile([128, 1024], F32, tag="sc", name="lead_ktqt")
            lv = [opp.tile([128, 520], F32, tag="oacc", name="lead_v01"),
                  pjp.tile([128, 512], F32, tag="pj", name="lead_v23")]

            def lead_mm(k):
                for half, (w_sb,) in enumerate([(wk_sb,), (wq_sb,)]):
                    nc.tensor.matmul(
                        lt[:, half * 512:(half + 1) * 512],
                        lhsT=w_sb[:, k * DHC: k * DHC + 128],
                        rhs=xT_sb[:, k * S: k * S + 512],
                        start=(k == 0),
                        stop=(k == KC - 1 and not with_bias),
                    )

            def lead_vmm(k, pair):
                ps = lv[pair]
                for h2 in range(2):
                    sc_ = 2 * pair + h2
                    nc.tensor.matmul(
                        ps[:, h2 * 256:(h2 + 1) * 256],
                        lhsT=xT_sb[:, k * S + sc_ * 128: k * S + (sc_ + 1) * 128],
                        rhs=wv_sb[:, k * DHC: k * DHC + DHC],
                        start=(k == 0 and h2 == 0),
                        stop=(k == KC - 1 and h2 == 1 and not with_bias),
                    )

            ramp(15)
            for k in range(KC):
                lead_mm(k)
                if k < KC - 1:
                    ramp(2 if k < 4 else 5)
            if with_bias:
                for half, bias in enumerate([bk, bq]):
                    nc.tensor.matmul(
                        lt[:, half * 512:(half + 1) * 512],
                        lhsT=bias[:, 0:128], rhs=ones16[0:1, :],
                        start=False, stop=True,
                    )
            nc.scalar.copy(kt_sb[:, 0:512], lt[:, 0:512])
            nc.vector.tensor_copy(qt_sb[:, 0:512], lt[:, 512:1024])

            # ---- schedule ----
            # Mandatory per-slot items (PV cadence, drains, transposes) are
            # placed at fixed global slots g = 16*w + c.  Everything else
            # (projection/V quarters, outproj pieces) lives in a strict-FIFO
            # unit queue packed greedily against each slot's leftover PE
            # budget, with earliest-slot gates and force-emit deadlines.
            sched = {}
            mcost = {}

            def at(g, fn, cost=0):
                sched.setdefault(g, []).append(fn)
                mcost[g] = mcost.get(g, 0) + cost

            # PV cadence: PV(w, c) lags exp by ~12 slots; the single O bank
            # serializes windows (drain_a frees it).  Last two windows
            # compress so the tail stays short.
            for w in range(NW):
                for c in range(SB):
                    if w < 6:
                        g = 16 * w + 12 + c
                    elif w == 6:
                        g = 108 + c if c < 8 else 112 + (c - 8) // 2
                    else:
                        g = (120 + c if c < 4 else
                             124 + (c - 4) // 2 if c < 12 else 200)
                    at(g, lambda w=w, c=c: pv(w, c), 240)
            for w in range(NW - 1):
                if w < 6:
                    ga, gtp = 16 * w + 27, 16 * w + 30
                else:
                    ga, gtp = 115, 118
                at(ga, lambda w=w: drain_a(w))
                at(ga + 1, lambda w=w: drain_b(w))
                for qc in range(4):
                    at(gtp + qc // 2, lambda w=w, qc=qc: drain_tp(w, qc), 60)

            # filler unit queue
            units = []

            def unit(e, dls, cost, mk):
                for q, d in enumerate(dls):
                    units.append((e[q] if isinstance(e, list) else e, d, cost,
                                  mk(q)))

            def mk_qk(dst, wt, bias, p, nt, eng="dve"):
                return lambda q: (lambda: qk_quarter(dst, wt, bias, p, nt, q,
                                                     eng))

            def mk_v(j):
                return lambda q: (lambda: v_quarter(j, q))

            def mk_po(qw, qc, n, eng="dve"):
                return lambda q: (lambda: outproj(qw, qc, n, eng))

            unit(0, [0, 1, 2, 3], 430, mk_qk(kt_sb, wk_sb, bk, 0, 1, "act"))
            unit(1, [3, 4, 5, 6], 430, mk_v(0))
            unit([1, 2, 3, 5], [4, 5, 6, 7], 430,
                 mk_qk(kt_sb, wk_sb, bk, 0, 2, "act"))
            unit(1, [5, 6, 7, 8], 430, mk_v(1))
            unit(0, [8, 9, 10, 11], 430, mk_v(2))
            unit([1, 2, 3, 5], [8, 9, 10, 11], 430,
                 mk_qk(kt_sb, wk_sb, bk, 0, 3, "act"))
            unit(0, [10, 11, 12, 13], 430, mk_v(3))
            unit(0, [12, 13, 14, 15], 430, mk_qk(qt_sb, wq_sb, bq, 0, 1))
            unit([3, 5, 5, 5], [14, 15, 16, 17], 430, mk_v(4))
            unit([4, 5, 5, 6], [18, 19, 20, 21], 430, mk_v(5))
            unit([4, 5, 6, 6], [20, 21, 22, 23], 430, mk_v(6))
            unit([5, 6, 6, 7], [22, 23, 24, 25], 430, mk_v(7))
            unit(0, [28, 29, 30, 31], 430, mk_qk(qt_sb, wq_sb, bq, 0, 2))
            unit(0, [44, 45, 46, 47], 430, mk_qk(qt_sb, wq_sb, bq, 0, 3))
            unit(0, [60, 61, 62, 63], 430, mk_qk(kt_sb, wk_sb, bk, 1, 0))
            unit(0, [64, 65, 66, 67], 430, mk_qk(kt_sb, wk_sb, bk, 1, 1))
            unit(0, [68, 69, 70, 71], 430, mk_qk(kt_sb, wk_sb, bk, 1, 2))
            unit(0, [72, 73, 74, 75], 430, mk_qk(kt_sb, wk_sb, bk, 1, 3))
            unit(0, [60, 61, 62, 63], 430, mk_qk(qt_sb, wq_sb, bq, 1, 0))
            unit(0, [76, 77, 78, 79], 430, mk_qk(qt_sb, wq_sb, bq, 1, 1))
            unit([88, 89, 90, 91], [92, 93, 94, 95], 430, mk_qk(qt_sb, wq_sb, bq, 1, 2))
            for i in range(8):
                units.append((96, 118 + i, 430, (lambda i=i: outproj(
                    0, i // 2, i % 2))))
            unit([100, 101, 102, 103], [108, 109, 110, 111], 430, mk_qk(qt_sb, wq_sb, bq, 1, 3))
            for i in range(8):
                units.append((112, 113 + i, 430, (lambda i=i: outproj(
                    1, i // 2, i % 2))))
            for i in range(8):
                units.append((120, 120 + i, 430, (lambda i=i: outproj(
                    2, i // 2, i % 2))))

            # ---- main loop with budget packing
            uidx = 0
            for w in range(NW):
                for c in range(SB):
                    g = 16 * w + c
                    emit_qk_exp(w, c)
                    for fn in sched.get(g, ()):
                        fn()
                    budget = 1038 - 430 - mcost.get(g, 0)
                    spent = 0
                    while uidx < len(units):
                        e, d, cost, fn = units[uidx]
                        if e > g:
                            break
                        if (d <= g or spent + cost <= budget + 120
                                or (spent == 0 and budget >= 300)):
                            fn()
                            spent += cost
                            uidx += 1
                        else:
                            break

            # ---- tail: leftover units, then finish window 7.  onmT copies
            # (DVE) go ahead of the outproj staging copies; staging copies
            # alternate ACT/DVE.
            for fn in sched.get(200, ()):
                fn()
            # direct-from-PSUM drain of window 7: normalization runs on ACT
            # (idle after the last exp), per-qc so transposes and outproj
            # pipeline behind it.
            O7 = otiles.pop(7)
            rc7 = rcp.tile([128, 8], F32, tag="rc", name="rc_7")
            nc.vector.reciprocal_approx_fast(out=rc7[:, :], in_=O7[:, 512:520])
            nrm7 = nrmp.tile([128, 512], BF, tag="nrm", name="nrm_7")
            CPY = mybir.ActivationFunctionType.Copy
            for qc in range(4):
                for h in range(2):
                    col = qc * 128 + h * 64
                    if h == 0:
                        nc.scalar.activation(
                            nrm7[:, col:col + 64], O7[:, col:col + 64], CPY,
                            scale=rc7[:, qc * 2 + h: qc * 2 + h + 1])
                    else:
                        nc.vector.tensor_scalar_mul(
                            nrm7[:, col:col + 64], O7[:, col:col + 64],
                            rc7[:, qc * 2 + h: qc * 2 + h + 1])
                tp7 = pjp.tile([128, 128], BF, tag="pj", name=f"tp_7_{qc}")
                nc.tensor.transpose(
                    tp7[:, :], nrm7[:, qc * 128:(qc + 1) * 128], ident[:, :])
                nc.vector.tensor_copy(
                    onmT_sb[:, S + 3 * 512 + qc * 128: S + 3 * 512 +
                            (qc + 1) * 128], tp7[:, :])
                po2 = scp.tile([128, 1024], F32, tag="sc",
                               name=f"po3_{qc}")
                t = 12 + qc
                for n in range(2):
                    for hp in range(2):
                        nc.tensor.matmul(
                            po2[:, n * 512:(n + 1) * 512],
                            lhsT=onmT_sb[:, hp * S + 3 * 512 + qc * 128:
                                         hp * S + 3 * 512 + (qc + 1) * 128],
                            rhs=wo_sb[:, hp * D + n * 512:
                                      hp * D + (n + 1) * 512],
                            start=(hp == 0), stop=(hp == 1),
                        )
                for n in range(2):
                    ot = otp.tile([128, 512], BF, tag="ot",
                                  name=f"ot3_{qc}_{n}")
                    if (qc + n) % 2 == 0:
                        nc.scalar.copy(ot[:, :], po2[:, n * 512:(n + 1) * 512])
                    else:
                        nc.vector.tensor_copy(
                            ot[:, :], po2[:, n * 512:(n + 1) * 512])
                    nc.sync.dma_start(
                        out_d[t * 128:(t + 1) * 128, n * 512:(n + 1) * 512],
                        ot[:, :])
            while uidx < len(units):
                units[uidx][3]()
                uidx += 1

    nc.compile()
    return nc


def _get_nc(with_bias=False):
    if with_bias not in _nc_cache:
        _nc_cache[with_bias] = _build_bass(with_bias=with_bias)
    return _nc_cache[with_bias]


def _prepare_in_maps(x, wq, bq, wk, bk, wv, bv, wo):
    import ml_dtypes

    bf16 = ml_dtypes.bfloat16
    x = np.asarray(x, np.float32)
    wq, bq = np.asarray(wq, np.float32), np.asarray(bq, np.float32)
    wk, bk = np.asarray(wk, np.float32), np.asarray(bk, np.float32)
    wv, bv = np.asarray(wv, np.float32), np.asarray(bv, np.float32)
    wo = np.asarray(wo, np.float32)

    xT = [np.ascontiguousarray(x[b].T).astype(bf16) for b in range(B)]
    in_maps = []
    for c in range(NC):
        b, j = divmod(c, HPC)
        cs = slice(DHC * j, DHC * (j + 1))
        bias3 = np.concatenate([bq[cs], bk[cs], bv[cs]]).reshape(1, 3 * DHC).astype(bf16)
        in_maps.append(
            {
                "xT": xT[b],
                "wq_c": np.ascontiguousarray(wq[:, cs]).astype(bf16),
                "wk_c": np.ascontiguousarray(wk[:, cs]).astype(bf16),
                "wv_c": np.ascontiguousarray(wv[:, cs]).astype(bf16),
                "wo_c": np.ascontiguousarray(wo[cs, :]).astype(bf16),
                "bias3": np.ascontiguousarray(bias3),
            }
        )
    return in_maps


def _gather(parts, bo):
    bo = np.asarray(bo, np.float32)
    out = np.empty((B, S, D), np.float32)
    for b in range(B):
        acc = np.asarray(parts[HPC * b], np.float32)
        for j in range(1, HPC):
            acc = acc + np.asarray(parts[HPC * b + j], np.float32)
        out[b] = acc + bo
    return out


def kernel(x, wq, bq, wk, bk, wv, bv, wo, bo):
    from concourse import bass_utils

    in_maps = _prepare_in_maps(x, wq, bq, wk, bk, wv, bv, wo)
    with_bias = bool(
        np.any(np.asarray(bq)) or np.any(np.asarray(bk)) or np.any(np.asarray(bv))
    )
    res = bass_utils.run_bass_kernel_spmd(
        nc=_get_nc(with_bias), in_maps=in_maps, core_ids=list(range(NC))
    )
    parts = [np.asarray(r["out"], np.float32) for r in res.results]
    return _gather(parts, bo)
